# revision 63
# baseline (speedup 1.0000x reference)
"""Trainium2 Bass kernel for nn_AttentionBlock (B=4, N=1024, C=1024, H=16).

Sharding: 8 cores = 4 batches x 2 head-groups (8 heads each). Each core
computes its batch's tokens for its 8 heads end-to-end (fused qkv+delta
projection, qk-LayerNorm, RoPE, softmax attention with value-residual mix,
and a partial output projection over its head columns). The host sums the
two partial projections per batch.

Measured ~202us on 8 trn2 cores (baseline lineage: 452us f32r ->
253us bf16 -> 202us this version). Key optimizations:
- all matmul operands bf16; LN mean-centering folded into the q/k weight
  columns on the host (exact: (x@W)@C = x@(W@C) with C = I - J/64), so the
  in-kernel LN is just rstd scaling (fp8 DoubleRow was tried and rejected:
  attention-weight/v quantization error does NOT average down — it scales
  with the same sqrt(sum a^2) factor as the signal -> ~5e-2 rel err);
- rope/vres tables are host-permuted to the exact SBUF layout so their
  DMAs are contiguous (128 descriptors, not 8192) and ride the idle
  scalar queue; w streams q+k columns before v columns (separate wv tile
  to avoid false chunk-row deps), early chunks alternate queues;
- score matmuls for the two 64-dim head-halves live in ONE [128,2,512]
  psum tile (their WAR clears atomically, so the scheduler keeps the A/B
  interleave) and run concurrently via tile_position row tiling;
- chunk-grained attention pipeline: per step one sc pair + one 1024-wide
  exp, av matmuls trail by 6 steps, psS bufs=2 gives the exp stream one
  step of slack; psV bufs=3 plus an early raw-av copy (normalize multiply
  reads the copy) so av psum banks free ~1.4us after their last matmul;
- warmup matmuls during the DMA-wait head keep the PE HAM clock at 8/8;
  the GpSimd broadcast ucode library is prewarmed behind the input DMA
  triggers (first use otherwise costs ~7.7us on the gpsimd queue);
- softmax normalize: ones-column denominator row -> GpSimd broadcast ->
  DVE approx-reciprocal -> one multiply; the Exp ACT table is preloaded
  during phase A so no table switch gates the first attention exp;
- tail: qh1 output-projection units are partial-summed (cc 0..2) during
  the last pair — two via SBUF f32 partials (DVE add) and two resident in
  retired score psum banks (ScalarE copy) — so only one matmul plus one
  copy/add per unit remains after the final softmax finale.
"""
import os
import sys

sys.path.insert(0, "/opt/trn_rl_repo")

import numpy as np
import ml_dtypes

import concourse.bass as bass
import concourse.bacc as bacc
import concourse.tile as tile
from concourse import mybir
from concourse.bass_utils import run_bass_kernel_spmd
from concourse.masks import make_identity

F32 = mybir.dt.float32
BF16 = mybir.dt.bfloat16

B, N, C, H = 4, 1024, 1024, 16
DH = C // H            # 64
HD = DH // 2           # 32
HPC = 8                # heads per core
NT = N // 128          # 8 token tiles
KC = (2 * C) // 128    # 16 contraction chunks for fused qkv+dt
EPS = 1e-5
AX = mybir.AxisListType.X
ALU = mybir.AluOpType
AF = mybir.ActivationFunctionType


def _bcast_free(ap, n, axis_pos=1):
    """Insert a step-0 free dim of size n at axis_pos of an AP."""
    new = list(ap.ap)
    new.insert(axis_pos, [0, n])
    return bass.AP(tensor=ap.tensor, offset=ap.offset, ap=new)


def _bcast_part(ap, n):
    """Partition-broadcast AP (step-0 partition dim) for DMA use."""
    return bass.AP(tensor=ap.tensor, offset=ap.offset, ap=[[0, n]] + list(ap.ap[1:]))


def build(lamb1, has_bias, has_ln, debug=False):
    """Build the single-core SPMD program.

    lamb1: python float (v-residual own-value weight; the residual weight
    lamb2 is folded into the host-prescaled vres input).
    has_bias: combined qkv+dt bias is nonzero -> biasd input present.
    has_ln: any qk-LayerNorm affine param nontrivial -> lnp input present.
    """
    nc = bacc.Bacc("TRN2", target_bir_lowering=False)

    xdT = nc.dram_tensor("xdT", [2 * C, N], BF16, kind="ExternalInput")
    w = nc.dram_tensor("w", [2 * C, 3 * HPC * DH], BF16, kind="ExternalInput")
    # vres / rope tables come host-permuted into SBUF layout (p-major).
    vres = nc.dram_tensor("vres", [128, NT * HPC * DH], BF16, kind="ExternalInput")
    ropec = nc.dram_tensor("ropec", [128, NT * DH], BF16, kind="ExternalInput")
    ropes = nc.dram_tensor("ropes", [128, NT * DH], BF16, kind="ExternalInput")
    wproj = nc.dram_tensor("wproj", [HPC * DH, C], BF16, kind="ExternalInput")
    biasd = None
    if has_bias:
        biasd = nc.dram_tensor("biasd", [1, 3 * HPC * DH], F32, kind="ExternalInput")
    lnpd = None
    if has_ln:
        lnpd = nc.dram_tensor("lnp", [4, DH], BF16, kind="ExternalInput")
    out = nc.dram_tensor("out", [N, C], BF16, kind="ExternalOutput")
    dbg = {}
    if debug:
        for nm, shp, dt in [
                ("d_qr", [N, HPC * DH], BF16), ("d_kr", [N, HPC * DH], BF16),
                ("d_kT", [128, N], BF16), ("d_qT", [128, N], BF16),
                ("d_v", [N, HPC * (DH + 1)], BF16),
                ("d_ex", [128, 1024], BF16), ("d_av", [DH + 1, 512], F32),
                ("d_rcp", [1, 512], F32), ("d_rep", [DH, 512], F32),
                ("d_outT", [128, N], BF16)]:
            dbg[nm] = nc.dram_tensor(nm, shp, dt, kind="ExternalOutput")

    with tile.TileContext(nc) as tc:
        with (
            tc.tile_pool(name="const", bufs=1) as constp,
            tc.tile_pool(name="longp", bufs=1) as longp,
        ):
            ident = constp.tile([128, 128], BF16)
            make_identity(nc, ident)
            eps_t = constp.tile([128, 1], F32)
            nc.vector.memset(eps_t, EPS)
            wsrc = constp.tile([128, 128], BF16)
            nc.vector.memset(wsrc, 0.5)
            ones_r = constp.tile([1, DH], F32)
            nc.vector.memset(ones_r, 1.0)

            bias_sb = None
            if biasd is not None:
                bias_sb = constp.tile([128, 3 * HPC * DH], F32)
                nc.scalar.dma_start(out=bias_sb, in_=_bcast_part(biasd[:, :], 128))
            ln_sb = None
            if lnpd is not None:
                ln_sb = constp.tile([128, 4, DH], BF16)
                nc.scalar.dma_start(out=ln_sb, in_=_bcast_part(lnpd[:, :], 128))

            # big persistent SBUF tensors
            xdT_sb = longp.tile([128, KC, N], BF16)
            w_sb = longp.tile([128, KC, 2 * HPC * DH], BF16)
            wv_sb = longp.tile([128, KC, HPC * DH], BF16)
            rpc_sb = longp.tile([128, NT, DH], BF16)
            rps_sb = longp.tile([128, NT, DH], BF16)
            vres_sb = longp.tile([128, NT, HPC, DH], BF16)
            v_sb = longp.tile([128, NT, HPC, DH + 1], BF16)
            qT_sb = longp.tile([128, HPC // 2, N], BF16)
            kT_sb = longp.tile([128, HPC // 2, N], BF16)
            outT_sb = longp.tile([128, HPC // 2, N], BF16)
            wproj_sb = longp.tile([128, 4, C], BF16)

            # ---- input DMAs. Critical order: the tiny rope tables and the
            # first xdT / w(q+k cols) chunks lead; v-projection columns,
            # vres and wproj stream later (first needed mid-phase-A).
            # sync queue: xdT + rope; gpsimd queue: w. ~5MB each.
            nc.sync.dma_start(out=xdT_sb[:, 0, 0:256], in_=xdT[0:128, 0:256])
            nc.gpsimd.dma_start(out=w_sb[:, 0, 0:1024], in_=w[0:128, 0:1024])
            nc.scalar.dma_start(out=rpc_sb,
                                in_=ropec[:, :].rearrange("p (t d) -> p t d", t=NT))
            nc.scalar.dma_start(out=rps_sb,
                                in_=ropes[:, :].rearrange("p (t d) -> p t d", t=NT))
            nc.sync.dma_start(out=xdT_sb[:, 0, 256:N], in_=xdT[0:128, 256:N])
            # first chunks alternate queues so the DMA-paced q-bulk start
            # gets both tensors' early chunks at ~2x cadence
            for kc in range(1, 5):
                qa, qb = ((nc.sync, nc.gpsimd) if kc % 2 == 1
                          else (nc.gpsimd, nc.sync))
                qa.dma_start(out=xdT_sb[:, kc, :],
                             in_=xdT[kc * 128:(kc + 1) * 128, :])
                qb.dma_start(out=w_sb[:, kc, 0:1024],
                             in_=w[kc * 128:(kc + 1) * 128, 0:1024])
            for kc in range(5, KC):
                nc.sync.dma_start(out=xdT_sb[:, kc, :],
                                  in_=xdT[kc * 128:(kc + 1) * 128, :])
                nc.gpsimd.dma_start(out=w_sb[:, kc, 0:1024],
                                    in_=w[kc * 128:(kc + 1) * 128, 0:1024])
            # late tensors: v columns of w, vres, wproj
            nc.scalar.dma_start(
                out=vres_sb,
                in_=vres[:, :].rearrange("p (t h d) -> p t h d", t=NT, h=HPC))
            for kc in range(KC):
                nc.gpsimd.dma_start(out=wv_sb[:, kc, :],
                                    in_=w[kc * 128:(kc + 1) * 128, 1024:1536])
            for cc in range(4):
                (nc.scalar if cc % 2 == 0 else nc.gpsimd).dma_start(
                    out=wproj_sb[:, cc, :], in_=wproj[cc * 128:(cc + 1) * 128, :])
            # prewarm the GpSimd custom-op library (first partition_broadcast
            # otherwise pays a ~7.7us ucode load right when the first softmax
            # finale needs it). The src reads a wproj_sb cell so the scheduler
            # cannot hoist it ahead of the input DMA triggers on the gpsimd
            # queue — the ucode load blocks that queue for its duration.
            gwarm = constp.tile([DH, 1], BF16)
            nc.gpsimd.partition_broadcast(gwarm[:, :], wproj_sb[0:1, 3, 0:1],
                                          channels=DH)

            # ones column of v (denominator row of the av matmul)
            nc.vector.memset(v_sb[:, :, :, DH:DH + 1], 1.0)

            # ------------- phase A: fused qkv+dt projection, LN, rope ------
            # q runs kc-outer (tracks DMA chunk arrival); k and v run t-outer
            # with inline postprocessing so PSUM tiles free progressively.
            # Each projection tile is first copied to bf16 SBUF by ScalarE —
            # that copy is the tile's only PSUM reader, so the bank frees
            # ~1us after the matmuls finish, and the LN/rope math runs on
            # fast packed-bf16 SBUF DVE ops. The q/k weights are host-
            # centered, so LN needs no mean subtraction here.
            with (
                tc.tile_pool(name="qkp", bufs=1) as qkp,
                tc.tile_pool(name="psA", bufs=8, space="PSUM") as psA,
                tc.tile_pool(name="scrA", bufs=3) as scrA,
                tc.tile_pool(name="stat", bufs=4) as stat,
            ):
                qr_sb = qkp.tile([128, NT, HPC, DH], BF16)
                kr_sb = qkp.tile([128, NT, HPC, DH], BF16)

                # HAM warmup: keep the PE busy during the DMA-wait head so
                # the clock gate is at 8/8 when the real matmuls arrive.
                # HAM warmup so the real matmuls (first DMA-gated, ~9-10us
                # in) start at the full 2.4 GHz clock. Fed by a memset tile
                # (no make_identity dependency) so it starts ~6us in.
                warm = psA.tile([128, 512], F32, name="warm", tag="pp")
                for _ in range(24):
                    nc.tensor.matmul(warm[:, 0:128], wsrc[:], wsrc[:],
                                     start=True, stop=True)

                def post_qk1(ps, t, ob, on_dve=False):
                    """PSUM-freeing copy only — emitted inline with the
                    projection matmuls so banks release at copy pace."""
                    ps3 = ps.rearrange("p (h d) -> p h d", h=HPC)
                    if bias_sb is not None:
                        nc.vector.tensor_add(
                            ps[:], ps[:], bias_sb[:, ob * 512:(ob + 1) * 512])
                    xb = scrA.tile([128, HPC, DH], BF16, tag="xb", bufs=17)
                    if on_dve:
                        nc.vector.tensor_copy(xb[:], ps3)
                    else:
                        nc.scalar.activation(out=xb[:], in_=ps3, func=AF.Copy)
                    return xb

                def post_qk2(xb, t, ob):
                    """LN rstd + rope chain (DVE), deferred past the copies."""
                    sqb = scrA.tile([128, HPC, DH], BF16, tag="sqb")
                    nc.vector.tensor_mul(sqb[:], xb[:], xb[:])
                    red_q = stat.tile([128, HPC], F32, tag="red_q")
                    nc.vector.reduce_sum(out=red_q[:], in_=sqb[:], axis=AX)
                    # rstd = 1/sqrt(sum(q^2)/DH + eps); Sqrt keeps one ACT
                    # table set in phase A, DVE reciprocal leaves Exp's
                    # table untouched for the attention phase.
                    rstd = stat.tile([128, HPC], F32, tag="rstd")
                    nc.scalar.activation(out=rstd[:], in_=red_q[:], func=AF.Sqrt,
                                         scale=1.0 / DH, bias=eps_t[:])
                    nc.vector.reciprocal(rstd[:], rstd[:])
                    xr = scrA.tile([128, HPC, DH], BF16, tag="xr")
                    nc.vector.tensor_tensor(
                        out=xr[:], in0=xb[:], in1=_bcast_free(rstd[:], DH, 2)[:],
                        op=ALU.mult)
                    if ln_sb is not None:
                        gi, bi = (0, 1) if ob == 0 else (2, 3)
                        nc.vector.tensor_tensor(
                            out=xr[:], in0=xr[:],
                            in1=_bcast_free(ln_sb[:, gi, :], HPC, 1)[:],
                            op=ALU.mult)
                        nc.vector.tensor_tensor(
                            out=xr[:], in0=xr[:],
                            in1=_bcast_free(ln_sb[:, bi, :], HPC, 1)[:],
                            op=ALU.add)
                    # rope: dst = xr*cos + rot_half(xr)*sin'
                    dst = (qr_sb if ob == 0 else kr_sb)
                    rpc = _bcast_free(rpc_sb[:, t], HPC, 1)
                    rps_lo = _bcast_free(rps_sb[:, t, 0:HD], HPC, 1)
                    rps_hi = _bcast_free(rps_sb[:, t, HD:DH], HPC, 1)
                    tc_ = scrA.tile([128, HPC, DH], BF16, tag="tc")
                    nc.vector.tensor_tensor(out=tc_[:], in0=xr[:],
                                            in1=rpc[:], op=ALU.mult)
                    tm = scrA.tile([128, HPC, DH], BF16, tag="tm")
                    nc.vector.tensor_tensor(out=tm[:, :, 0:HD],
                                            in0=xr[:, :, HD:DH],
                                            in1=rps_lo[:], op=ALU.mult)
                    nc.vector.tensor_tensor(out=tm[:, :, HD:DH],
                                            in0=xr[:, :, 0:HD],
                                            in1=rps_hi[:], op=ALU.mult)
                    nc.vector.tensor_tensor(out=dst[:, t], in0=tc_[:],
                                            in1=tm[:], op=ALU.add)

                def post_v(ps, t, on_dve):
                    ps3 = ps.rearrange("p (h d) -> p h d", h=HPC)
                    if bias_sb is not None:
                        nc.vector.tensor_add(
                            ps[:], ps[:], bias_sb[:, 1024:1536])
                    if on_dve:
                        # attention-phase tiles: keep ScalarE free for exps
                        nc.vector.tensor_scalar_mul(
                            v_sb[:, t, :, 0:DH], in0=ps3, scalar1=float(lamb1))
                    else:
                        nc.scalar.activation(out=v_sb[:, t, :, 0:DH], in_=ps3,
                                             func=AF.Copy, scale=float(lamb1))
                    nc.vector.tensor_tensor(out=v_sb[:, t, :, 0:DH],
                                            in0=v_sb[:, t, :, 0:DH],
                                            in1=vres_sb[:, t], op=ALU.add)

                def proj_mms(ps, ob, t, kcs):
                    for kc in kcs:
                        wsl = (wv_sb[:, kc, :] if ob == 2
                               else w_sb[:, kc, ob * 512:(ob + 1) * 512])
                        nc.tensor.matmul(
                            ps[:],
                            xdT_sb[:, kc, t * 128:(t + 1) * 128],
                            wsl,
                            start=(kc == 0), stop=(kc == KC - 1))

                def proj_tile(ob, t, pool, tag):
                    ps = pool.tile([128, 512], F32, name=f"pt{ob}_{t}",
                                   tag=tag)
                    proj_mms(ps, ob, t, range(KC))
                    return ps

                def transpose_ob(src, dstT):
                    for j in range(HPC // 2):
                        tp = psA.tile([128, 512], F32, tag="pp")
                        tpb = tp.bitcast(BF16)
                        for t in range(NT):
                            nc.tensor.transpose(
                                tpb[:, t * 128:(t + 1) * 128],
                                src[:, t, 2 * j:2 * j + 2, :]
                                   .rearrange("p h d -> p (h d)"),
                                ident[:])
                        if dstT is qT_sb:
                            nc.scalar.activation(out=dstT[:, j, :], in_=tpb[:],
                                                 func=AF.Copy)
                        else:
                            nc.vector.tensor_copy(dstT[:, j, :], tpb[:])

                # q: bulk kc-outer (tracks DMA chunk arrival), then a
                # t-outer tail with inline posts so the DVE postprocess
                # stream is spread instead of bursting 8 chains at once.
                KS = KC
                ps_tiles = [psA.tile([128, 512], F32, name=f"pp{_t}",
                                     tag="pp")
                            for _t in range(NT)]
                for kc in range(KS):
                    for t in range(NT):
                        nc.tensor.matmul(
                            ps_tiles[t][:],
                            xdT_sb[:, kc, t * 128:(t + 1) * 128],
                            w_sb[:, kc, 0:512],
                            start=(kc == 0), stop=False)
                q_xb = []
                for t in range(NT):
                    for kc in range(KS, KC):
                        nc.tensor.matmul(
                            ps_tiles[t][:],
                            xdT_sb[:, kc, t * 128:(t + 1) * 128],
                            w_sb[:, kc, 0:512],
                            start=False, stop=(kc == KC - 1))
                    # DVE is idle here (part2 chains deferred), so alternate
                    # the copies across engines for 2x bank-release pace
                    q_xb.append(post_qk1(ps_tiles[t], t, 0,
                                         on_dve=(t % 2 == 1)))
                for t in range(NT):
                    post_qk2(q_xb[t], t, 0)
                # k (copies on ScalarE — DVE is draining the q chains),
                # then both transposes, then the first half of v
                k_xb = []
                for t in range(NT):
                    k_xb.append(post_qk1(proj_tile(1, t, psA, "pp"), t, 1))
                for t in range(NT):
                    post_qk2(k_xb[t], t, 1)
                # v tiles before the transposes: independent PE work covers
                # the DVE draining the k postprocess chains, so the
                # transposes (which need every k tile's rope done) run
                # gap-free right before attention consumes them.
                transpose_ob(qr_sb, qT_sb)
                for t in range(6):
                    post_v(proj_tile(2, t, psA, "pp"), t, on_dve=False)
                transpose_ob(kr_sb, kT_sb)
                # preload the Exp ACT table (no more Sqrts follow) so the
                # ~2.7us table switch overlaps the v tiles, not the first
                # attention exp.
                nc.scalar.activation(out=eps_t[:], in_=eps_t[:], func=AF.Exp)
                if debug:
                    rr2 = "(t p) (h d) -> p t h d"
                    nc.sync.dma_start(
                        out=dbg["d_qr"][:, :].rearrange(rr2, p=128, h=HPC),
                        in_=qr_sb)
                    nc.sync.dma_start(
                        out=dbg["d_kr"][:, :].rearrange(rr2, p=128, h=HPC),
                        in_=kr_sb)
                    nc.sync.dma_start(out=dbg["d_qT"][:, :], in_=qT_sb[:, 0, :])
                    nc.sync.dma_start(out=dbg["d_kT"][:, :], in_=kT_sb[:, 0, :])

            # ------------- attention + interleaved fillers ------------------
            # (j, qh) pair-major iteration covering both 64-dim head-halves.
            # The two halves' score matmuls are interleaved at tile_position
            # rows 0/64 so the PE row-tiles them concurrently. PE filler
            # between attention chunks: first the deferred second half of
            # the v projection (tiles 4-7, emitted in 8-matmul halves), then
            # out-projection tiles once a query half's finales land.
            with (
                tc.tile_pool(name="psS", bufs=2, space="PSUM") as psS,
                tc.tile_pool(name="psV", bufs=3, space="PSUM") as psV,
                tc.tile_pool(name="psP", bufs=1, space="PSUM") as psP,
                tc.tile_pool(name="expp", bufs=8) as expp,
                tc.tile_pool(name="nrm", bufs=2) as nrm,
                tc.tile_pool(name="outp", bufs=2) as outp,
            ):
                pairs = [(j, qh) for qh in range(2) for j in range(HPC // 2)]
                NSTEP = NT // 2  # kk steps per pair (2 key chunks per half)
                av_t = {}
                scale = 1.0 / float(np.sqrt(DH))

                def emit_sc_pair(pi, kc):
                    # one chunk-pair per step: a single [128,2,512] psum tile
                    # holds BOTH head-halves' score chunks, so their WAR
                    # clears atomically — the scheduler keeps the A/B
                    # interleave and the PE row-tiles the two 64-contraction
                    # matmuls concurrently. bufs=2 gives one step of slack
                    # between the sc matmuls and the exp of the prior step.
                    j, qh = pairs[pi]
                    sc = psS.tile([128, 2, 512], F32, tag="sc", bufs=2)
                    qs = slice(qh * 512, (qh + 1) * 512)
                    ks = slice(kc * 128, (kc + 1) * 128)
                    nc.tensor.matmul(
                        sc[:, 0, :], kT_sb[0:DH, j, ks], qT_sb[0:DH, j, qs],
                        start=True, stop=True, tile_position=(0, 0))
                    nc.tensor.matmul(
                        sc[:, 1, :], kT_sb[DH:128, j, ks],
                        qT_sb[DH:128, j, qs],
                        start=True, stop=True, tile_position=(DH, 0))
                    ex = expp.tile([128, 2, 512], BF16, tag="ex", bufs=8)
                    if pi == len(pairs) - 1:
                        # final pair: per-half exps so the drain chain
                        # (exp -> av -> finale -> out-proj) starts sooner
                        for i in range(2):
                            nc.scalar.activation(out=ex[:, i, :],
                                                 in_=sc[:, i, :],
                                                 func=AF.Exp, scale=scale)
                    else:
                        nc.scalar.activation(out=ex[:], in_=sc[:],
                                             func=AF.Exp, scale=scale)
                    if debug and pi == 0 and kc == 0:
                        nc.sync.dma_start(
                            out=dbg["d_ex"][:, 0:1024].rearrange(
                                "p (a b) -> p a b", a=2),
                            in_=ex[:])
                    return ex

                def emit_av_pair(pi, kc, ex):
                    j, qh = pairs[pi]
                    if kc == 0:
                        av_t[(pi, 0)] = psV.tile([DH + 1, 512], F32,
                                                 name=f"av{pi}_0", tag="av")
                        av_t[(pi, 1)] = psV.tile([DH + 1, 512], F32,
                                                 name=f"av{pi}_1", tag="av")
                    nc.tensor.matmul(
                        av_t[(pi, 0)][:], v_sb[:, kc, 2 * j, :],
                        ex[:, 0, :], start=(kc == 0), stop=(kc == NT - 1))
                    nc.tensor.matmul(
                        av_t[(pi, 1)][:], v_sb[:, kc, 2 * j + 1, :],
                        ex[:, 1, :], start=(kc == 0), stop=(kc == NT - 1))

                def emit_finale(pi, hh):
                    j, qh = pairs[pi]
                    ro = 64 * hh
                    av = av_t.pop((pi, hh))
                    rep = nrm.tile([DH, 2, 512], F32, tag="rep")
                    # sums row (psum partition 64) -> partition 0 SBUF on
                    # DVE (ScalarE is saturated with exps), then gpsimd-
                    # broadcast to 64 partitions, then approx-recip there
                    # (the custom DVE op misbehaves at base >= 64).
                    sums = nrm.tile([1, 512], F32, tag="sums")
                    if pi == len(pairs) - 1:
                        # ScalarE is free once the last exps retire; taking
                        # the sums copy there shortens the serialized DVE
                        # chain that gates the final out-proj units
                        nc.scalar.activation(out=sums[:], in_=av[DH:DH + 1, :],
                                             func=AF.Copy)
                    else:
                        nc.vector.tensor_copy(sums[:], av[DH:DH + 1, :])
                    # raw-av copy frees the psum bank ~1.4us after the last
                    # av matmul (instead of after the whole 3us normalize
                    # chain) so the next pair's av allocation never stalls.
                    avr = nrm.tile([DH, 512], F32, tag="avr", bufs=3)
                    nc.vector.tensor_copy(avr[:], av[0:DH, :])
                    nc.gpsimd.partition_broadcast(
                        rep[:, 0, :], sums[:], channels=DH)
                    nc.vector.reciprocal_approx_fast(
                        out=rep[:, 1, :], in_=rep[:, 0, :])
                    nc.vector.tensor_tensor(
                        out=outT_sb[ro:ro + DH, j, qh * 512:(qh + 1) * 512],
                        in0=avr[:], in1=rep[:, 1, :], op=ALU.mult)
                    if debug and pi == 0 and hh == 0:
                        avc = nrm.tile([DH + 1, 512], F32, tag="avc")
                        nc.vector.tensor_copy(avc[:], av[:])
                        nc.sync.dma_start(out=dbg["d_av"][:, :], in_=avc)
                        nc.sync.dma_start(out=dbg["d_rcp"][:, :],
                                          in_=rep[0:1, 1, :])
                        nc.sync.dma_start(out=dbg["d_rep"][:, :],
                                          in_=rep[:, 1, :])

                # qh0 units (t 0..3) run fully as attention fillers. qh1
                # units (t 4..7) run in two stages: cc 0..2 partial-summed
                # to SBUF f32 during the last pair (j0..2 finales are in),
                # then one cc=3 matmul + DVE add after the last finale.
                proj_units = [(t, oh) for t in range(NT) for oh in range(2)]
                stg_t = {}
                prt = {}
                state = {"emitted": 0, "finales": 0, "vdef": 0, "partial": 8}

                def unit_mms(pp, t, oh, ccs, start_cc, stop_cc):
                    for cc in ccs:
                        nc.tensor.matmul(
                            pp[:],
                            outT_sb[:, cc, t * 128:(t + 1) * 128],
                            wproj_sb[:, cc, oh * 512:(oh + 1) * 512],
                            start=(cc == start_cc), stop=(cc == stop_cc))

                def emit_proj_unit():
                    t, oh = proj_units[state["emitted"]]
                    state["emitted"] += 1
                    if oh == 0:
                        stg_t[t] = outp.tile([128, C], BF16, name=f"stg{t}",
                                             tag="stg")
                    pp = psP.tile([128, 512], F32, tag="pp2")
                    unit_mms(pp, t, oh, range(4), 0, 3)
                    nc.vector.tensor_copy(
                        stg_t[t][:, oh * 512:(oh + 1) * 512], pp[:])
                    if oh == 1:
                        nc.sync.dma_start(out=out[t * 128:(t + 1) * 128, :],
                                          in_=stg_t.pop(t))

                def emit_partial_unit():
                    # t=4,5 units: cc0..2 partial -> SBUF f32 (final = DVE add)
                    t, oh = proj_units[state["partial"]]
                    state["partial"] += 1
                    pp = psP.tile([128, 512], F32, tag="pp2")
                    unit_mms(pp, t, oh, range(3), 0, 2)
                    prt[(t, oh)] = outp.tile([128, 512], F32,
                                             name=f"prt{t}_{oh}", tag="prt",
                                             bufs=4)
                    nc.vector.tensor_copy(prt[(t, oh)][:], pp[:])

                prt_ps = {}

                def emit_psum_partials():
                    # t=6,7 units: cc0..2 stay resident in psS tiles (their
                    # sc traffic is over); final = one accumulating matmul +
                    # a ScalarE copy, so the tail splits across DVE + ScalarE
                    for t in (6, 7):
                        pt = psS.tile([128, 2, 512], F32, name=f"prtps{t}",
                                      tag="sc", bufs=2)
                        for oh in range(2):
                            unit_mms(pt[:, oh, :], t, oh, range(3), 0, -1)
                        prt_ps[t] = pt

                def emit_final_unit(t, oh):
                    if oh == 0:
                        stg_t[t] = outp.tile([128, C], BF16, name=f"stg{t}",
                                             tag="stg")
                    if t in prt_ps:
                        pt = prt_ps[t]
                        unit_mms(pt[:, oh, :], t, oh, [3], -1, 3)
                        nc.scalar.activation(
                            out=stg_t[t][:, oh * 512:(oh + 1) * 512],
                            in_=pt[:, oh, :], func=AF.Copy)
                    else:
                        # the av banks are all retired by now — rotating the
                        # final-unit psums through psV avoids serializing on
                        # the single psP bank
                        pp = psV.tile([128, 512], F32, tag="av",
                                      name=f"fu{t}_{oh}")
                        unit_mms(pp, t, oh, [3], 3, 3)
                        nc.vector.tensor_tensor(
                            out=stg_t[t][:, oh * 512:(oh + 1) * 512],
                            in0=pp[:], in1=prt.pop((t, oh))[:], op=ALU.add)
                    if oh == 1:
                        nc.sync.dma_start(out=out[t * 128:(t + 1) * 128, :],
                                          in_=stg_t.pop(t))

                # deferred v tiles 4-7 spread over attention steps 0-5;
                # deferred v tiles 4-7 spread as half-tile fillers over the
                # first chunk-steps; tile T's second half lands at step
                # 2(T-4)+1, well before its first av reader (kc=T) is
                # emitted at step T+6 with the lookahead of 6.
                vplan = [(6, 0), (6, 1), (7, 0), (7, 1)]
                vps = {}

                def emit_filler():
                    if state["vdef"] < len(vplan):
                        t, half = vplan[state["vdef"]]
                        state["vdef"] += 1
                        if half == 0:
                            vps[t] = psP.tile([128, 512], F32,
                                              name=f"vt{t}", tag="pp2")
                            proj_mms(vps[t], 2, t, range(0, KC // 2))
                        else:
                            proj_mms(vps[t], 2, t, range(KC // 2, KC))
                            post_v(vps.pop(t), t, on_dve=True)
                        return
                    if state["finales"] >= 8 and state["emitted"] < 8:
                        emit_proj_unit()
                        return
                    if state["finales"] >= 14 and state["partial"] < 12:
                        emit_partial_unit()
                        if state["partial"] < 12:
                            emit_partial_unit()

                # pipeline: sc/exp run 6 chunk-steps ahead of av; finales
                # (all DVE/GpSimd) are emitted as soon as the last av lands.
                steps = [(pi, kc) for pi in range(len(pairs))
                         for kc in range(NT)]
                exq = []      # (pi, kc, ex) awaiting av emission
                fill_tick = 0
                for (pi, kc) in steps:
                    # last pair: drop the av lookahead to 1 so the drain
                    # chain (av -> finale -> out-proj tail) starts sooner
                    look = 1 if pi == len(pairs) - 1 else 6
                    while len(exq) >= look:
                        api, akc, aex = exq.pop(0)
                        emit_av_pair(api, akc, aex)
                        if akc == NT - 1:
                            emit_finale(api, 0)
                            emit_finale(api, 1)
                            state["finales"] += 2
                    exq.append((pi, kc, emit_sc_pair(pi, kc)))
                    # fillers are ~1.7-1us of PE work; one per two chunk-
                    # steps keeps the PE just above the exp pace
                    fill_tick += 1
                    if state["vdef"] < len(vplan) or fill_tick % 2 == 0:
                        emit_filler()
                for (api, akc, aex) in exq:
                    emit_av_pair(api, akc, aex)
                    if akc == NT - 1:
                        emit_finale(api, 0)
                        emit_finale(api, 1)
                        state["finales"] += 2
                if debug:
                    nc.sync.dma_start(out=dbg["d_outT"][:, :],
                                      in_=outT_sb[:, 0, :])
                    nc.sync.dma_start(
                        out=dbg["d_v"][:, :].rearrange(
                            "(t p) (h d) -> p t h d", p=128, h=HPC),
                        in_=v_sb)
                while state["emitted"] < 8:
                    emit_proj_unit()
                while state["partial"] < 12:
                    emit_partial_unit()
                emit_psum_partials()
                # interleave DVE-add units (t4,5) with ScalarE-copy units
                # (t6,7) so the tail splits across both engines
                for t in (4, 6, 5, 7):
                    emit_final_unit(t, 0)
                    emit_final_unit(t, 1)

    nc.finalize()
    return nc


_CACHE = {}
_LAST_RES = None


def _bf16(a):
    return np.ascontiguousarray(a.astype(ml_dtypes.bfloat16))


def kernel(x, rope, delta_t_emb, v_residual_v1, Wqkv, bqkv, Wdt, bdt,
           qn_g, qn_b, kn_g, kn_b, lamb1, lamb2, Wproj, bproj):
    x = np.asarray(x, np.float32)
    rope = np.ascontiguousarray(np.asarray(rope, np.float32))
    delta_t_emb = np.asarray(delta_t_emb, np.float32)
    v_residual_v1 = np.asarray(v_residual_v1, np.float32)
    Wqkv = np.asarray(Wqkv, np.float32)
    Wdt = np.asarray(Wdt, np.float32)
    Wproj = np.asarray(Wproj, np.float32)
    bias = np.asarray(bqkv, np.float32) + np.asarray(bdt, np.float32)
    l1 = float(np.asarray(lamb1)); l2 = float(np.asarray(lamb2))
    qn_g = np.asarray(qn_g, np.float32); qn_b = np.asarray(qn_b, np.float32)
    kn_g = np.asarray(kn_g, np.float32); kn_b = np.asarray(kn_b, np.float32)

    has_bias = bool(np.any(bias))
    has_ln = not (np.all(qn_g == 1.0) and np.all(qn_b == 0.0)
                  and np.all(kn_g == 1.0) and np.all(kn_b == 0.0))

    dbgf = bool(int(os.environ.get("KERNEL_DEBUG", "0")))
    key = (l1, has_bias, has_ln, dbgf)
    if key not in _CACHE:
        _CACHE[key] = build(l1, has_bias, has_ln, debug=dbgf)
    nc = _CACHE[key]

    # host-prepared rope tables in SBUF layout [p, t*DH]:
    # cos table and sign-folded sin table (rotate_half absorbed:
    # out = x*cos + rot(x)*sin' with sin' = [-sin_lo || sin_hi]).
    sin = rope[:, 0:DH]; cos = rope[:, DH:2 * DH]
    sinp = np.concatenate([-sin[:, 0:HD], sin[:, HD:DH]], axis=1)

    def _ptile(a):  # [N, DH] -> [128, NT*DH] with n = t*128 + p
        return _bf16(a.reshape(NT, 128, DH).transpose(1, 0, 2).reshape(128, -1))

    cos_p = _ptile(cos)
    sin_p = _ptile(sinp)

    in_maps = []
    for c in range(8):
        b = c // 2
        g = c % 2
        rsl = slice(g * 512, (g + 1) * 512)
        w_core = np.concatenate([
            np.concatenate([Wqkv[rsl], Wqkv[C:][rsl], Wqkv[2 * C:][rsl]], 0).T,
            np.concatenate([Wdt[rsl], Wdt[C:][rsl], Wdt[2 * C:][rsl]], 0).T,
        ], axis=0)
        w_core = np.ascontiguousarray(w_core)
        bc = np.concatenate([bias[rsl], bias[C:][rsl], bias[2 * C:][rsl]])
        bc = bc.astype(np.float32).copy()
        # fold LN mean-centering into the q/k weight+bias head blocks
        # (exact: (x@W + b)@C = x@(W@C) + b@C with C = I - J/64)
        for ob in range(2):
            for h in range(HPC):
                sl = slice(ob * 512 + h * DH, ob * 512 + (h + 1) * DH)
                w_core[:, sl] -= w_core[:, sl].mean(axis=1, keepdims=True)
                bc[sl] -= bc[sl].mean()
        # vres in SBUF layout [p, t, h, d] flattened
        vr = (l2 * v_residual_v1[b, g * 8:(g + 1) * 8]).transpose(1, 0, 2)
        vr = vr.reshape(NT, 128, HPC, DH).transpose(1, 0, 2, 3).reshape(128, -1)
        m = {
            "xdT": _bf16(np.concatenate([x[b].T, delta_t_emb[b].T], 0)),
            "w": _bf16(w_core),
            "vres": _bf16(vr),
            "ropec": cos_p,
            "ropes": sin_p,
            "wproj": _bf16(Wproj[:, rsl].T),
        }
        if has_bias:
            m["biasd"] = np.ascontiguousarray(bc[None, :].astype(np.float32))
        if has_ln:
            m["lnp"] = _bf16(np.stack([qn_g, qn_b, kn_g, kn_b], 0))
        in_maps.append(m)

    trace = bool(int(os.environ.get("KERNEL_TRACE", "0")))
    res = run_bass_kernel_spmd(nc, in_maps, core_ids=list(range(8)), trace=trace)
    global _LAST_RES
    _LAST_RES = res
    if trace and res.exec_time_ns is not None:
        print(f"HW exec time: {res.exec_time_ns} ns")
        kernel.last_exec_time_ns = res.exec_time_ns
        kernel.last_results = res

    out = np.empty((B, N, C), np.float32)
    for b in range(B):
        out[b] = (res.results[2 * b]["out"].astype(np.float32)
                  + res.results[2 * b + 1]["out"].astype(np.float32))
    bproj = np.asarray(bproj, np.float32)
    if np.any(bproj):
        out += bproj[None, None, :]
    return out


# revision 67
# speedup vs baseline: 1.1573x; 1.1573x over previous
"""Trainium2 Bass kernel for nn_AttentionBlock (B=4, N=1024, C=1024, H=16).

Sharding: 8 cores = 4 batches x 2 head-groups (8 heads each). Each core
computes its batch's tokens for its 8 heads end-to-end (fused qkv+delta
projection, qk-LayerNorm, RoPE, softmax attention with value-residual mix,
and a partial output projection over its head columns). The host sums the
two partial projections per batch.

Measured ~202us on 8 trn2 cores (baseline lineage: 452us f32r ->
253us bf16 -> 202us this version). Key optimizations:
- all matmul operands bf16; LN mean-centering folded into the q/k weight
  columns on the host (exact: (x@W)@C = x@(W@C) with C = I - J/64), so the
  in-kernel LN is just rstd scaling (fp8 DoubleRow was tried and rejected:
  attention-weight/v quantization error does NOT average down — it scales
  with the same sqrt(sum a^2) factor as the signal -> ~5e-2 rel err);
- rope/vres tables are host-permuted to the exact SBUF layout so their
  DMAs are contiguous (128 descriptors, not 8192) and ride the idle
  scalar queue; w streams q+k columns before v columns (separate wv tile
  to avoid false chunk-row deps), early chunks alternate queues;
- score matmuls for the two 64-dim head-halves live in ONE [128,2,512]
  psum tile (their WAR clears atomically, so the scheduler keeps the A/B
  interleave) and run concurrently via tile_position row tiling;
- chunk-grained attention pipeline: per step one sc pair + one 1024-wide
  exp, av matmuls trail by 6 steps, psS bufs=2 gives the exp stream one
  step of slack; psV bufs=3 plus an early raw-av copy (normalize multiply
  reads the copy) so av psum banks free ~1.4us after their last matmul;
- warmup matmuls during the DMA-wait head keep the PE HAM clock at 8/8;
  the GpSimd broadcast ucode library is prewarmed behind the input DMA
  triggers (first use otherwise costs ~7.7us on the gpsimd queue);
- softmax normalize: ones-column denominator row -> GpSimd broadcast ->
  DVE approx-reciprocal -> one multiply; the Exp ACT table is preloaded
  during phase A so no table switch gates the first attention exp;
- tail: qh1 output-projection units are partial-summed (cc 0..2) during
  the last pair — two via SBUF f32 partials (DVE add) and two resident in
  retired score psum banks (ScalarE copy) — so only one matmul plus one
  copy/add per unit remains after the final softmax finale.
"""
import os
import sys

sys.path.insert(0, "/opt/trn_rl_repo")

import numpy as np
import ml_dtypes

import concourse.bass as bass
import concourse.bacc as bacc
import concourse.tile as tile
from concourse import mybir
from concourse.bass_utils import run_bass_kernel_spmd
from concourse.masks import make_identity

F32 = mybir.dt.float32
BF16 = mybir.dt.bfloat16

B, N, C, H = 4, 1024, 1024, 16
DH = C // H            # 64
HD = DH // 2           # 32
HPC = 8                # heads per core
NT = N // 128          # 8 token tiles
KC = (2 * C) // 128    # 16 contraction chunks for fused qkv+dt
EPS = 1e-5
AX = mybir.AxisListType.X
ALU = mybir.AluOpType
AF = mybir.ActivationFunctionType


def _bcast_free(ap, n, axis_pos=1):
    """Insert a step-0 free dim of size n at axis_pos of an AP."""
    new = list(ap.ap)
    new.insert(axis_pos, [0, n])
    return bass.AP(tensor=ap.tensor, offset=ap.offset, ap=new)


def _bcast_part(ap, n):
    """Partition-broadcast AP (step-0 partition dim) for DMA use."""
    return bass.AP(tensor=ap.tensor, offset=ap.offset, ap=[[0, n]] + list(ap.ap[1:]))


def build(lamb1, has_bias, has_ln, debug=False):
    """Build the single-core SPMD program.

    lamb1: python float (v-residual own-value weight; the residual weight
    lamb2 is folded into the host-prescaled vres input).
    has_bias: combined qkv+dt bias is nonzero -> biasd input present.
    has_ln: any qk-LayerNorm affine param nontrivial -> lnp input present.
    """
    nc = bacc.Bacc("TRN2", target_bir_lowering=False)

    xdT = nc.dram_tensor("xdT", [2 * C, N], BF16, kind="ExternalInput")
    w = nc.dram_tensor("w", [2 * C, 3 * HPC * DH], BF16, kind="ExternalInput")
    # vres / rope tables come host-permuted into SBUF layout (p-major).
    vres = nc.dram_tensor("vres", [128, NT * HPC * DH], BF16, kind="ExternalInput")
    ropec = nc.dram_tensor("ropec", [128, NT * DH], BF16, kind="ExternalInput")
    ropes = nc.dram_tensor("ropes", [128, NT * DH], BF16, kind="ExternalInput")
    wproj = nc.dram_tensor("wproj", [HPC * DH, C], BF16, kind="ExternalInput")
    biasd = None
    if has_bias:
        biasd = nc.dram_tensor("biasd", [1, 3 * HPC * DH], F32, kind="ExternalInput")
    lnpd = None
    if has_ln:
        lnpd = nc.dram_tensor("lnp", [4, DH], BF16, kind="ExternalInput")
    out = nc.dram_tensor("out", [N, C], BF16, kind="ExternalOutput")
    dbg = {}
    if debug:
        for nm, shp, dt in [
                ("d_qr", [N, HPC * DH], BF16), ("d_kr", [N, HPC * DH], BF16),
                ("d_kT", [128, N], BF16), ("d_qT", [128, N], BF16),
                ("d_v", [N, HPC * (DH + 1)], BF16),
                ("d_ex", [128, 1024], BF16), ("d_av", [DH + 1, 512], F32),
                ("d_rcp", [1, 512], F32), ("d_rep", [DH, 512], F32),
                ("d_outT", [128, N], BF16)]:
            dbg[nm] = nc.dram_tensor(nm, shp, dt, kind="ExternalOutput")

    with tile.TileContext(nc) as tc:
        with (
            tc.tile_pool(name="const", bufs=1) as constp,
            tc.tile_pool(name="longp", bufs=1) as longp,
        ):
            ident = constp.tile([128, 128], BF16)
            make_identity(nc, ident)
            eps_t = constp.tile([128, 1], F32)
            nc.vector.memset(eps_t, EPS)
            wsrc = constp.tile([128, 128], BF16)
            nc.vector.memset(wsrc, 0.5)
            ones_r = constp.tile([1, DH], F32)
            nc.vector.memset(ones_r, 1.0)

            bias_sb = None
            if biasd is not None:
                bias_sb = constp.tile([128, 3 * HPC * DH], F32)
                nc.scalar.dma_start(out=bias_sb, in_=_bcast_part(biasd[:, :], 128))
            ln_sb = None
            if lnpd is not None:
                ln_sb = constp.tile([128, 4, DH], BF16)
                nc.scalar.dma_start(out=ln_sb, in_=_bcast_part(lnpd[:, :], 128))

            # big persistent SBUF tensors
            xdT_sb = longp.tile([128, KC, N], BF16)
            w_sb = longp.tile([128, KC, 2 * HPC * DH], BF16)
            wv_sb = longp.tile([128, KC, HPC * DH], BF16)
            rpc_sb = longp.tile([128, NT, DH], BF16)
            rps_sb = longp.tile([128, NT, DH], BF16)
            vres_sb = longp.tile([128, NT, HPC, DH], BF16)
            v_sb = longp.tile([128, NT, HPC, DH + 1], BF16)
            qT_sb = longp.tile([128, HPC // 2, N], BF16)
            kT_sb = longp.tile([128, HPC // 2, N], BF16)
            outT_sb = longp.tile([128, HPC // 2, N], BF16)
            wproj_sb = longp.tile([128, 4, C], BF16)

            # ---- input DMAs. Critical order: the tiny rope tables and the
            # first xdT / w(q+k cols) chunks lead; v-projection columns,
            # vres and wproj stream later (first needed mid-phase-A).
            # sync queue: xdT + rope; gpsimd queue: w. ~5MB each.
            nc.sync.dma_start(out=xdT_sb[:, 0, 0:256], in_=xdT[0:128, 0:256])
            nc.gpsimd.dma_start(out=w_sb[:, 0, 0:1024], in_=w[0:128, 0:1024])
            nc.scalar.dma_start(out=rpc_sb,
                                in_=ropec[:, :].rearrange("p (t d) -> p t d", t=NT))
            nc.scalar.dma_start(out=rps_sb,
                                in_=ropes[:, :].rearrange("p (t d) -> p t d", t=NT))
            nc.sync.dma_start(out=xdT_sb[:, 0, 256:N], in_=xdT[0:128, 256:N])
            # first chunks alternate queues so the DMA-paced q-bulk start
            # gets both tensors' early chunks at ~2x cadence
            for kc in range(1, 5):
                qa, qb = ((nc.sync, nc.gpsimd) if kc % 2 == 1
                          else (nc.gpsimd, nc.sync))
                qa.dma_start(out=xdT_sb[:, kc, :],
                             in_=xdT[kc * 128:(kc + 1) * 128, :])
                qb.dma_start(out=w_sb[:, kc, 0:1024],
                             in_=w[kc * 128:(kc + 1) * 128, 0:1024])
            for kc in range(5, KC):
                nc.sync.dma_start(out=xdT_sb[:, kc, :],
                                  in_=xdT[kc * 128:(kc + 1) * 128, :])
                nc.gpsimd.dma_start(out=w_sb[:, kc, 0:1024],
                                    in_=w[kc * 128:(kc + 1) * 128, 0:1024])
            # late tensors: v columns of w, vres, wproj
            nc.scalar.dma_start(
                out=vres_sb,
                in_=vres[:, :].rearrange("p (t h d) -> p t h d", t=NT, h=HPC))
            for kc in range(KC):
                nc.gpsimd.dma_start(out=wv_sb[:, kc, :],
                                    in_=w[kc * 128:(kc + 1) * 128, 1024:1536])
            for cc in range(4):
                (nc.scalar if cc % 2 == 0 else nc.gpsimd).dma_start(
                    out=wproj_sb[:, cc, :], in_=wproj[cc * 128:(cc + 1) * 128, :])
            # prewarm the GpSimd custom-op library (first partition_broadcast
            # otherwise pays a ~7.7us ucode load right when the first softmax
            # finale needs it). The src reads a wproj_sb cell so the scheduler
            # cannot hoist it ahead of the input DMA triggers on the gpsimd
            # queue — the ucode load blocks that queue for its duration.
            gwarm = constp.tile([DH, 1], BF16)
            nc.gpsimd.partition_broadcast(gwarm[:, :], wproj_sb[0:1, 3, 0:1],
                                          channels=DH)

            # ones column of v (denominator row of the av matmul)
            nc.vector.memset(v_sb[:, :, :, DH:DH + 1], 1.0)

            # ------------- phase A: fused qkv+dt projection, LN, rope ------
            # q runs kc-outer (tracks DMA chunk arrival); k and v run t-outer
            # with inline postprocessing so PSUM tiles free progressively.
            # Each projection tile is first copied to bf16 SBUF by ScalarE —
            # that copy is the tile's only PSUM reader, so the bank frees
            # ~1us after the matmuls finish, and the LN/rope math runs on
            # fast packed-bf16 SBUF DVE ops. The q/k weights are host-
            # centered, so LN needs no mean subtraction here.
            with (
                tc.tile_pool(name="qkp", bufs=1) as qkp,
                tc.tile_pool(name="psA", bufs=8, space="PSUM") as psA,
                tc.tile_pool(name="scrA", bufs=3) as scrA,
                tc.tile_pool(name="stat", bufs=4) as stat,
            ):
                qr_sb = qkp.tile([128, NT, HPC, DH], BF16)
                kr_sb = qkp.tile([128, NT, HPC, DH], BF16)

                # HAM warmup: keep the PE busy during the DMA-wait head so
                # the clock gate is at 8/8 when the real matmuls arrive.
                # HAM warmup so the real matmuls (first DMA-gated, ~9-10us
                # in) start at the full 2.4 GHz clock. Fed by a memset tile
                # (no make_identity dependency) so it starts ~6us in.
                warm = psA.tile([128, 512], F32, name="warm", tag="pp")
                for _ in range(24):
                    nc.tensor.matmul(warm[:, 0:128], wsrc[:], wsrc[:],
                                     start=True, stop=True)

                def post_qk1(ps, t, ob, on_dve=False):
                    """PSUM-freeing copy only — emitted inline with the
                    projection matmuls so banks release at copy pace."""
                    ps3 = ps.rearrange("p (h d) -> p h d", h=HPC)
                    if bias_sb is not None:
                        nc.vector.tensor_add(
                            ps[:], ps[:], bias_sb[:, ob * 512:(ob + 1) * 512])
                    xb = scrA.tile([128, HPC, DH], BF16, tag="xb", bufs=17)
                    if on_dve:
                        nc.vector.tensor_copy(xb[:], ps3)
                    else:
                        nc.scalar.activation(out=xb[:], in_=ps3, func=AF.Copy)
                    return xb

                def post_qk2(xb, t, ob):
                    """LN rstd + rope chain (DVE), deferred past the copies."""
                    sqb = scrA.tile([128, HPC, DH], BF16, tag="sqb")
                    nc.vector.tensor_mul(sqb[:], xb[:], xb[:])
                    red_q = stat.tile([128, HPC], F32, tag="red_q")
                    nc.vector.reduce_sum(out=red_q[:], in_=sqb[:], axis=AX)
                    # rstd = 1/sqrt(sum(q^2)/DH + eps); Sqrt keeps one ACT
                    # table set in phase A, DVE reciprocal leaves Exp's
                    # table untouched for the attention phase.
                    rstd = stat.tile([128, HPC], F32, tag="rstd")
                    nc.scalar.activation(out=rstd[:], in_=red_q[:], func=AF.Sqrt,
                                         scale=1.0 / DH, bias=eps_t[:])
                    nc.vector.reciprocal(rstd[:], rstd[:])
                    xr = scrA.tile([128, HPC, DH], BF16, tag="xr")
                    nc.vector.tensor_tensor(
                        out=xr[:], in0=xb[:], in1=_bcast_free(rstd[:], DH, 2)[:],
                        op=ALU.mult)
                    if ln_sb is not None:
                        gi, bi = (0, 1) if ob == 0 else (2, 3)
                        nc.vector.tensor_tensor(
                            out=xr[:], in0=xr[:],
                            in1=_bcast_free(ln_sb[:, gi, :], HPC, 1)[:],
                            op=ALU.mult)
                        nc.vector.tensor_tensor(
                            out=xr[:], in0=xr[:],
                            in1=_bcast_free(ln_sb[:, bi, :], HPC, 1)[:],
                            op=ALU.add)
                    # rope: dst = xr*cos + rot_half(xr)*sin'
                    dst = (qr_sb if ob == 0 else kr_sb)
                    rpc = _bcast_free(rpc_sb[:, t], HPC, 1)
                    rps_lo = _bcast_free(rps_sb[:, t, 0:HD], HPC, 1)
                    rps_hi = _bcast_free(rps_sb[:, t, HD:DH], HPC, 1)
                    tc_ = scrA.tile([128, HPC, DH], BF16, tag="tc")
                    nc.vector.tensor_tensor(out=tc_[:], in0=xr[:],
                                            in1=rpc[:], op=ALU.mult)
                    tm = scrA.tile([128, HPC, DH], BF16, tag="tm")
                    nc.vector.tensor_tensor(out=tm[:, :, 0:HD],
                                            in0=xr[:, :, HD:DH],
                                            in1=rps_lo[:], op=ALU.mult)
                    nc.vector.tensor_tensor(out=tm[:, :, HD:DH],
                                            in0=xr[:, :, 0:HD],
                                            in1=rps_hi[:], op=ALU.mult)
                    nc.vector.tensor_tensor(out=dst[:, t], in0=tc_[:],
                                            in1=tm[:], op=ALU.add)

                def post_v(ps, t, on_dve):
                    ps3 = ps.rearrange("p (h d) -> p h d", h=HPC)
                    if bias_sb is not None:
                        nc.vector.tensor_add(
                            ps[:], ps[:], bias_sb[:, 1024:1536])
                    if on_dve:
                        # attention-phase tiles: keep ScalarE free for exps
                        nc.vector.tensor_scalar_mul(
                            v_sb[:, t, :, 0:DH], in0=ps3, scalar1=float(lamb1))
                    else:
                        nc.scalar.activation(out=v_sb[:, t, :, 0:DH], in_=ps3,
                                             func=AF.Copy, scale=float(lamb1))
                    nc.vector.tensor_tensor(out=v_sb[:, t, :, 0:DH],
                                            in0=v_sb[:, t, :, 0:DH],
                                            in1=vres_sb[:, t], op=ALU.add)

                def proj_mms(ps, ob, t, kcs):
                    for kc in kcs:
                        wsl = (wv_sb[:, kc, :] if ob == 2
                               else w_sb[:, kc, ob * 512:(ob + 1) * 512])
                        nc.tensor.matmul(
                            ps[:],
                            xdT_sb[:, kc, t * 128:(t + 1) * 128],
                            wsl,
                            start=(kc == 0), stop=(kc == KC - 1))

                def proj_tile(ob, t, pool, tag):
                    ps = pool.tile([128, 512], F32, name=f"pt{ob}_{t}",
                                   tag=tag)
                    proj_mms(ps, ob, t, range(KC))
                    return ps

                def transpose_ob(src, dstT):
                    for j in range(HPC // 2):
                        tp = psA.tile([128, 512], F32, tag="pp")
                        tpb = tp.bitcast(BF16)
                        for t in range(NT):
                            nc.tensor.transpose(
                                tpb[:, t * 128:(t + 1) * 128],
                                src[:, t, 2 * j:2 * j + 2, :]
                                   .rearrange("p h d -> p (h d)"),
                                ident[:])
                        if dstT is qT_sb:
                            nc.scalar.activation(out=dstT[:, j, :], in_=tpb[:],
                                                 func=AF.Copy)
                        else:
                            nc.vector.tensor_copy(dstT[:, j, :], tpb[:])

                # q: bulk kc-outer (tracks DMA chunk arrival), then a
                # t-outer tail with inline posts so the DVE postprocess
                # stream is spread instead of bursting 8 chains at once.
                KS = KC
                ps_tiles = [psA.tile([128, 512], F32, name=f"pp{_t}",
                                     tag="pp")
                            for _t in range(NT)]
                for kc in range(KS):
                    for t in range(NT):
                        nc.tensor.matmul(
                            ps_tiles[t][:],
                            xdT_sb[:, kc, t * 128:(t + 1) * 128],
                            w_sb[:, kc, 0:512],
                            start=(kc == 0), stop=False)
                q_xb = []
                for t in range(NT):
                    for kc in range(KS, KC):
                        nc.tensor.matmul(
                            ps_tiles[t][:],
                            xdT_sb[:, kc, t * 128:(t + 1) * 128],
                            w_sb[:, kc, 0:512],
                            start=False, stop=(kc == KC - 1))
                    # DVE is idle here (part2 chains deferred), so alternate
                    # the copies across engines for 2x bank-release pace
                    q_xb.append(post_qk1(ps_tiles[t], t, 0,
                                         on_dve=(t % 2 == 1)))
                for t in range(NT):
                    post_qk2(q_xb[t], t, 0)
                # k (copies on ScalarE — DVE is draining the q chains),
                # then both transposes, then the first half of v
                k_xb = []
                for t in range(NT):
                    k_xb.append(post_qk1(proj_tile(1, t, psA, "pp"), t, 1))
                for t in range(NT):
                    post_qk2(k_xb[t], t, 1)
                # v tiles before the transposes: independent PE work covers
                # the DVE draining the k postprocess chains, so the
                # transposes (which need every k tile's rope done) run
                # gap-free right before attention consumes them.
                transpose_ob(qr_sb, qT_sb)
                for t in range(6):
                    post_v(proj_tile(2, t, psA, "pp"), t, on_dve=False)
                transpose_ob(kr_sb, kT_sb)
                # preload the Exp ACT table (no more Sqrts follow) so the
                # ~2.7us table switch overlaps the v tiles, not the first
                # attention exp.
                nc.scalar.activation(out=eps_t[:], in_=eps_t[:], func=AF.Exp)
                if debug:
                    rr2 = "(t p) (h d) -> p t h d"
                    nc.sync.dma_start(
                        out=dbg["d_qr"][:, :].rearrange(rr2, p=128, h=HPC),
                        in_=qr_sb)
                    nc.sync.dma_start(
                        out=dbg["d_kr"][:, :].rearrange(rr2, p=128, h=HPC),
                        in_=kr_sb)
                    nc.sync.dma_start(out=dbg["d_qT"][:, :], in_=qT_sb[:, 0, :])
                    nc.sync.dma_start(out=dbg["d_kT"][:, :], in_=kT_sb[:, 0, :])

            # ------------- attention + interleaved fillers ------------------
            # (j, qh) pair-major iteration covering both 64-dim head-halves.
            # The two halves' score matmuls are interleaved at tile_position
            # rows 0/64 so the PE row-tiles them concurrently. PE filler
            # between attention chunks: first the deferred second half of
            # the v projection (tiles 4-7, emitted in 8-matmul halves), then
            # out-projection tiles once a query half's finales land.
            with (
                tc.tile_pool(name="psS", bufs=2, space="PSUM") as psS,
                tc.tile_pool(name="psV", bufs=3, space="PSUM") as psV,
                tc.tile_pool(name="psP", bufs=1, space="PSUM") as psP,
                tc.tile_pool(name="expp", bufs=8) as expp,
                tc.tile_pool(name="nrm", bufs=2) as nrm,
                tc.tile_pool(name="outp", bufs=2) as outp,
            ):
                pairs = [(j, qh) for qh in range(2) for j in range(HPC // 2)]
                NSTEP = NT // 2  # kk steps per pair (2 key chunks per half)
                av_t = {}
                scale = 1.0 / float(np.sqrt(DH))

                def emit_sc_pair(pi, kc):
                    # one chunk-pair per step: a single [128,2,512] psum tile
                    # holds BOTH head-halves' score chunks, so their WAR
                    # clears atomically — the scheduler keeps the A/B
                    # interleave and the PE row-tiles the two 64-contraction
                    # matmuls concurrently. bufs=2 gives one step of slack
                    # between the sc matmuls and the exp of the prior step.
                    j, qh = pairs[pi]
                    sc = psS.tile([128, 2, 512], F32, tag="sc", bufs=2)
                    qs = slice(qh * 512, (qh + 1) * 512)
                    ks = slice(kc * 128, (kc + 1) * 128)
                    nc.tensor.matmul(
                        sc[:, 0, :], kT_sb[0:DH, j, ks], qT_sb[0:DH, j, qs],
                        start=True, stop=True, tile_position=(0, 0))
                    nc.tensor.matmul(
                        sc[:, 1, :], kT_sb[DH:128, j, ks],
                        qT_sb[DH:128, j, qs],
                        start=True, stop=True, tile_position=(DH, 0))
                    ex = expp.tile([128, 2, 512], BF16, tag="ex", bufs=8)
                    if pi == len(pairs) - 1:
                        # final pair: per-half exps so the drain chain
                        # (exp -> av -> finale -> out-proj) starts sooner
                        for i in range(2):
                            nc.scalar.activation(out=ex[:, i, :],
                                                 in_=sc[:, i, :],
                                                 func=AF.Exp, scale=scale)
                    else:
                        nc.scalar.activation(out=ex[:], in_=sc[:],
                                             func=AF.Exp, scale=scale)
                    if debug and pi == 0 and kc == 0:
                        nc.sync.dma_start(
                            out=dbg["d_ex"][:, 0:1024].rearrange(
                                "p (a b) -> p a b", a=2),
                            in_=ex[:])
                    return ex

                def emit_av_pair(pi, kc, ex):
                    j, qh = pairs[pi]
                    if kc == 0:
                        av_t[(pi, 0)] = psV.tile([DH + 1, 512], F32,
                                                 name=f"av{pi}_0", tag="av")
                        av_t[(pi, 1)] = psV.tile([DH + 1, 512], F32,
                                                 name=f"av{pi}_1", tag="av")
                    nc.tensor.matmul(
                        av_t[(pi, 0)][:], v_sb[:, kc, 2 * j, :],
                        ex[:, 0, :], start=(kc == 0), stop=(kc == NT - 1))
                    nc.tensor.matmul(
                        av_t[(pi, 1)][:], v_sb[:, kc, 2 * j + 1, :],
                        ex[:, 1, :], start=(kc == 0), stop=(kc == NT - 1))

                def emit_finale(pi, hh):
                    j, qh = pairs[pi]
                    ro = 64 * hh
                    av = av_t.pop((pi, hh))
                    rep = nrm.tile([DH, 2, 512], F32, tag="rep")
                    # sums row (psum partition 64) -> partition 0 SBUF on
                    # DVE (ScalarE is saturated with exps), then gpsimd-
                    # broadcast to 64 partitions, then approx-recip there
                    # (the custom DVE op misbehaves at base >= 64).
                    sums = nrm.tile([1, 512], F32, tag="sums")
                    if pi == len(pairs) - 1:
                        # ScalarE is free once the last exps retire; taking
                        # the sums copy there shortens the serialized DVE
                        # chain that gates the final out-proj units
                        nc.scalar.activation(out=sums[:], in_=av[DH:DH + 1, :],
                                             func=AF.Copy)
                    else:
                        nc.vector.tensor_copy(sums[:], av[DH:DH + 1, :])
                    # raw-av copy frees the psum bank ~1.4us after the last
                    # av matmul (instead of after the whole 3us normalize
                    # chain) so the next pair's av allocation never stalls.
                    avr = nrm.tile([DH, 512], F32, tag="avr", bufs=3)
                    nc.vector.tensor_copy(avr[:], av[0:DH, :])
                    nc.gpsimd.partition_broadcast(
                        rep[:, 0, :], sums[:], channels=DH)
                    nc.vector.reciprocal_approx_fast(
                        out=rep[:, 1, :], in_=rep[:, 0, :])
                    nc.vector.tensor_tensor(
                        out=outT_sb[ro:ro + DH, j, qh * 512:(qh + 1) * 512],
                        in0=avr[:], in1=rep[:, 1, :], op=ALU.mult)
                    if debug and pi == 0 and hh == 0:
                        avc = nrm.tile([DH + 1, 512], F32, tag="avc")
                        nc.vector.tensor_copy(avc[:], av[:])
                        nc.sync.dma_start(out=dbg["d_av"][:, :], in_=avc)
                        nc.sync.dma_start(out=dbg["d_rcp"][:, :],
                                          in_=rep[0:1, 1, :])
                        nc.sync.dma_start(out=dbg["d_rep"][:, :],
                                          in_=rep[:, 1, :])

                # qh0 units (t 0..3) run fully as attention fillers. qh1
                # units (t 4..7) run in two stages: cc 0..2 partial-summed
                # to SBUF f32 during the last pair (j0..2 finales are in),
                # then one cc=3 matmul + DVE add after the last finale.
                proj_units = [(t, oh) for t in range(NT) for oh in range(2)]
                stg_t = {}
                prt = {}
                state = {"emitted": 0, "finales": 0, "vdef": 0, "partial": 8}

                def unit_mms(pp, t, oh, ccs, start_cc, stop_cc):
                    for cc in ccs:
                        nc.tensor.matmul(
                            pp[:],
                            outT_sb[:, cc, t * 128:(t + 1) * 128],
                            wproj_sb[:, cc, oh * 512:(oh + 1) * 512],
                            start=(cc == start_cc), stop=(cc == stop_cc))

                def emit_proj_unit():
                    t, oh = proj_units[state["emitted"]]
                    state["emitted"] += 1
                    if oh == 0:
                        stg_t[t] = outp.tile([128, C], BF16, name=f"stg{t}",
                                             tag="stg")
                    pp = psP.tile([128, 512], F32, tag="pp2")
                    unit_mms(pp, t, oh, range(4), 0, 3)
                    nc.vector.tensor_copy(
                        stg_t[t][:, oh * 512:(oh + 1) * 512], pp[:])
                    if oh == 1:
                        nc.sync.dma_start(out=out[t * 128:(t + 1) * 128, :],
                                          in_=stg_t.pop(t))

                def emit_partial_unit():
                    # t=4,5 units: cc0..2 partial -> SBUF f32 (final = DVE add)
                    t, oh = proj_units[state["partial"]]
                    state["partial"] += 1
                    pp = psP.tile([128, 512], F32, tag="pp2")
                    unit_mms(pp, t, oh, range(3), 0, 2)
                    prt[(t, oh)] = outp.tile([128, 512], F32,
                                             name=f"prt{t}_{oh}", tag="prt",
                                             bufs=4)
                    nc.vector.tensor_copy(prt[(t, oh)][:], pp[:])

                prt_ps = {}

                def emit_psum_partials():
                    # t=6,7 units: cc0..2 stay resident in psS tiles (their
                    # sc traffic is over); final = one accumulating matmul +
                    # a ScalarE copy, so the tail splits across DVE + ScalarE
                    for t in (6, 7):
                        pt = psS.tile([128, 2, 512], F32, name=f"prtps{t}",
                                      tag="sc", bufs=2)
                        for oh in range(2):
                            unit_mms(pt[:, oh, :], t, oh, range(3), 0, -1)
                        prt_ps[t] = pt

                def emit_final_unit(t, oh):
                    if oh == 0:
                        stg_t[t] = outp.tile([128, C], BF16, name=f"stg{t}",
                                             tag="stg")
                    if t in prt_ps:
                        pt = prt_ps[t]
                        unit_mms(pt[:, oh, :], t, oh, [3], -1, 3)
                        nc.scalar.activation(
                            out=stg_t[t][:, oh * 512:(oh + 1) * 512],
                            in_=pt[:, oh, :], func=AF.Copy)
                    else:
                        # the av banks are all retired by now — rotating the
                        # final-unit psums through psV avoids serializing on
                        # the single psP bank
                        pp = psV.tile([128, 512], F32, tag="av",
                                      name=f"fu{t}_{oh}")
                        unit_mms(pp, t, oh, [3], 3, 3)
                        nc.vector.tensor_tensor(
                            out=stg_t[t][:, oh * 512:(oh + 1) * 512],
                            in0=pp[:], in1=prt.pop((t, oh))[:], op=ALU.add)
                    if oh == 1:
                        nc.sync.dma_start(out=out[t * 128:(t + 1) * 128, :],
                                          in_=stg_t.pop(t))

                # deferred v tiles 4-7 spread over attention steps 0-5;
                # deferred v tiles 4-7 spread as half-tile fillers over the
                # first chunk-steps; tile T's second half lands at step
                # 2(T-4)+1, well before its first av reader (kc=T) is
                # emitted at step T+6 with the lookahead of 6.
                vplan = [(6, 0), (6, 1), (7, 0), (7, 1)]
                vps = {}

                def emit_filler():
                    if state["vdef"] < len(vplan):
                        t, half = vplan[state["vdef"]]
                        state["vdef"] += 1
                        if half == 0:
                            vps[t] = psP.tile([128, 512], F32,
                                              name=f"vt{t}", tag="pp2")
                            proj_mms(vps[t], 2, t, range(0, KC // 2))
                        else:
                            proj_mms(vps[t], 2, t, range(KC // 2, KC))
                            post_v(vps.pop(t), t, on_dve=True)
                        return
                    if state["finales"] >= 8 and state["emitted"] < 8:
                        emit_proj_unit()
                        return
                    if state["finales"] >= 14 and state["partial"] < 12:
                        emit_partial_unit()
                        if state["partial"] < 12:
                            emit_partial_unit()

                # pipeline: sc/exp run 6 chunk-steps ahead of av; finales
                # (all DVE/GpSimd) are emitted as soon as the last av lands.
                steps = [(pi, kc) for pi in range(len(pairs))
                         for kc in range(NT)]
                exq = []      # (pi, kc, ex) awaiting av emission
                fill_tick = 0
                for (pi, kc) in steps:
                    # last pair: drop the av lookahead to 1 so the drain
                    # chain (av -> finale -> out-proj tail) starts sooner
                    look = 1 if pi == len(pairs) - 1 else 6
                    while len(exq) >= look:
                        api, akc, aex = exq.pop(0)
                        emit_av_pair(api, akc, aex)
                        if akc == NT - 1:
                            emit_finale(api, 0)
                            emit_finale(api, 1)
                            state["finales"] += 2
                    exq.append((pi, kc, emit_sc_pair(pi, kc)))
                    # fillers are ~1.7-1us of PE work; one per two chunk-
                    # steps keeps the PE just above the exp pace
                    fill_tick += 1
                    if state["vdef"] < len(vplan) or fill_tick % 2 == 0:
                        emit_filler()
                for (api, akc, aex) in exq:
                    emit_av_pair(api, akc, aex)
                    if akc == NT - 1:
                        emit_finale(api, 0)
                        emit_finale(api, 1)
                        state["finales"] += 2
                if debug:
                    nc.sync.dma_start(out=dbg["d_outT"][:, :],
                                      in_=outT_sb[:, 0, :])
                    nc.sync.dma_start(
                        out=dbg["d_v"][:, :].rearrange(
                            "(t p) (h d) -> p t h d", p=128, h=HPC),
                        in_=v_sb)
                while state["emitted"] < 8:
                    emit_proj_unit()
                while state["partial"] < 12:
                    emit_partial_unit()
                emit_psum_partials()
                # interleave DVE-add units (t4,5) with ScalarE-copy units
                # (t6,7) so the tail splits across both engines
                for t in (4, 6, 5, 7):
                    emit_final_unit(t, 0)
                    emit_final_unit(t, 1)

    nc.finalize()
    return nc


_CACHE = {}
_LAST_RES = None


def _bf16(a):
    return np.ascontiguousarray(a.astype(ml_dtypes.bfloat16))


def kernel(x, rope, delta_t_emb, v_residual_v1, Wqkv, bqkv, Wdt, bdt,
           qn_g, qn_b, kn_g, kn_b, lamb1, lamb2, Wproj, bproj):
    x = np.asarray(x, np.float32)
    rope = np.ascontiguousarray(np.asarray(rope, np.float32))
    delta_t_emb = np.asarray(delta_t_emb, np.float32)
    v_residual_v1 = np.asarray(v_residual_v1, np.float32)
    Wqkv = np.asarray(Wqkv, np.float32)
    Wdt = np.asarray(Wdt, np.float32)
    Wproj = np.asarray(Wproj, np.float32)
    bias = np.asarray(bqkv, np.float32) + np.asarray(bdt, np.float32)
    l1 = float(np.asarray(lamb1)); l2 = float(np.asarray(lamb2))
    qn_g = np.asarray(qn_g, np.float32); qn_b = np.asarray(qn_b, np.float32)
    kn_g = np.asarray(kn_g, np.float32); kn_b = np.asarray(kn_b, np.float32)

    has_bias = bool(np.any(bias))
    has_ln = not (np.all(qn_g == 1.0) and np.all(qn_b == 0.0)
                  and np.all(kn_g == 1.0) and np.all(kn_b == 0.0))

    dbgf = bool(int(os.environ.get("KERNEL_DEBUG", "0")))
    key = (l1, has_bias, has_ln, dbgf)
    if key not in _CACHE:
        _CACHE[key] = build(l1, has_bias, has_ln, debug=dbgf)
    nc = _CACHE[key]

    # host-prepared rope tables in SBUF layout [p, t*DH]:
    # cos table and sign-folded sin table (rotate_half absorbed:
    # out = x*cos + rot(x)*sin' with sin' = [-sin_lo || sin_hi]).
    sin = rope[:, 0:DH]; cos = rope[:, DH:2 * DH]
    sinp = np.concatenate([-sin[:, 0:HD], sin[:, HD:DH]], axis=1)

    def _ptile(a):  # [N, DH] -> [128, NT*DH] with n = t*128 + p
        return _bf16(a.reshape(NT, 128, DH).transpose(1, 0, 2).reshape(128, -1))

    cos_p = _ptile(cos)
    sin_p = _ptile(sinp)

    in_maps = []
    for c in range(8):
        b = c // 2
        g = c % 2
        rsl = slice(g * 512, (g + 1) * 512)
        w_core = np.concatenate([
            np.concatenate([Wqkv[rsl], Wqkv[C:][rsl], Wqkv[2 * C:][rsl]], 0).T,
            np.concatenate([Wdt[rsl], Wdt[C:][rsl], Wdt[2 * C:][rsl]], 0).T,
        ], axis=0)
        w_core = np.ascontiguousarray(w_core)
        bc = np.concatenate([bias[rsl], bias[C:][rsl], bias[2 * C:][rsl]])
        bc = bc.astype(np.float32).copy()
        # fold LN mean-centering into the q/k weight+bias head blocks
        # (exact: (x@W + b)@C = x@(W@C) + b@C with C = I - J/64)
        for ob in range(2):
            for h in range(HPC):
                sl = slice(ob * 512 + h * DH, ob * 512 + (h + 1) * DH)
                w_core[:, sl] -= w_core[:, sl].mean(axis=1, keepdims=True)
                bc[sl] -= bc[sl].mean()
        # vres in SBUF layout [p, t, h, d] flattened
        vr = (l2 * v_residual_v1[b, g * 8:(g + 1) * 8]).transpose(1, 0, 2)
        vr = vr.reshape(NT, 128, HPC, DH).transpose(1, 0, 2, 3).reshape(128, -1)
        m = {
            "xdT": _bf16(np.concatenate([x[b].T, delta_t_emb[b].T], 0)),
            "w": _bf16(w_core),
            "vres": _bf16(vr),
            "ropec": cos_p,
            "ropes": sin_p,
            "wproj": _bf16(Wproj[:, rsl].T),
        }
        if has_bias:
            m["biasd"] = np.ascontiguousarray(bc[None, :].astype(np.float32))
        if has_ln:
            m["lnp"] = _bf16(np.stack([qn_g, qn_b, kn_g, kn_b], 0))
        in_maps.append(m)

    trace = bool(int(os.environ.get("KERNEL_TRACE", "0")))
    res = run_bass_kernel_spmd(nc, in_maps, core_ids=list(range(8)), trace=trace)
    global _LAST_RES
    _LAST_RES = res
    if trace and res.exec_time_ns is not None:
        print(f"HW exec time: {res.exec_time_ns} ns")
        kernel.last_exec_time_ns = res.exec_time_ns
        kernel.last_results = res

    out = np.empty((B, N, C), np.float32)
    for b in range(B):
        out[b] = (res.results[2 * b]["out"].astype(np.float32)
                  + res.results[2 * b + 1]["out"].astype(np.float32))
    bproj = np.asarray(bproj, np.float32)
    if np.any(bproj):
        out += bproj[None, None, :]
    return out


# revision 69
# speedup vs baseline: 1.1602x; 1.0025x over previous
"""Trainium2 Bass kernel for nn_AttentionBlock (B=4, N=1024, C=1024, H=16).

Sharding: 8 cores = 4 batches x 2 head-groups (8 heads each). Each core
computes its batch's tokens for its 8 heads end-to-end (fused qkv+delta
projection, qk-LayerNorm, RoPE, softmax attention with value-residual mix,
and a partial output projection over its head columns). The host sums the
two partial projections per batch.

Measured ~202us on 8 trn2 cores (baseline lineage: 452us f32r ->
253us bf16 -> 202us this version). Key optimizations:
- all matmul operands bf16; LN mean-centering folded into the q/k weight
  columns on the host (exact: (x@W)@C = x@(W@C) with C = I - J/64), so the
  in-kernel LN is just rstd scaling (fp8 DoubleRow was tried and rejected:
  attention-weight/v quantization error does NOT average down — it scales
  with the same sqrt(sum a^2) factor as the signal -> ~5e-2 rel err);
- rope/vres tables are host-permuted to the exact SBUF layout so their
  DMAs are contiguous (128 descriptors, not 8192) and ride the idle
  scalar queue; w streams q+k columns before v columns (separate wv tile
  to avoid false chunk-row deps), early chunks alternate queues;
- score matmuls for the two 64-dim head-halves live in ONE [128,2,512]
  psum tile (their WAR clears atomically, so the scheduler keeps the A/B
  interleave) and run concurrently via tile_position row tiling;
- chunk-grained attention pipeline: per step one sc pair + one 1024-wide
  exp, av matmuls trail by 6 steps, psS bufs=2 gives the exp stream one
  step of slack; psV bufs=3 plus an early raw-av copy (normalize multiply
  reads the copy) so av psum banks free ~1.4us after their last matmul;
- warmup matmuls during the DMA-wait head keep the PE HAM clock at 8/8;
  the GpSimd broadcast ucode library is prewarmed behind the input DMA
  triggers (first use otherwise costs ~7.7us on the gpsimd queue);
- softmax normalize: ones-column denominator row -> GpSimd broadcast ->
  DVE approx-reciprocal -> one multiply; the Exp ACT table is preloaded
  during phase A so no table switch gates the first attention exp;
- tail: qh1 output-projection units are partial-summed (cc 0..2) during
  the last pair — two via SBUF f32 partials (DVE add) and two resident in
  retired score psum banks (ScalarE copy) — so only one matmul plus one
  copy/add per unit remains after the final softmax finale.
"""
import os
import sys

sys.path.insert(0, "/opt/trn_rl_repo")

import numpy as np
import ml_dtypes

import concourse.bass as bass
import concourse.bacc as bacc
import concourse.tile as tile
from concourse import mybir
from concourse.bass_utils import run_bass_kernel_spmd
from concourse.masks import make_identity

F32 = mybir.dt.float32
BF16 = mybir.dt.bfloat16

B, N, C, H = 4, 1024, 1024, 16
DH = C // H            # 64
HD = DH // 2           # 32
HPC = 8                # heads per core
NT = N // 128          # 8 token tiles
KC = (2 * C) // 128    # 16 contraction chunks for fused qkv+dt
EPS = 1e-5
AX = mybir.AxisListType.X
ALU = mybir.AluOpType
AF = mybir.ActivationFunctionType


def _bcast_free(ap, n, axis_pos=1):
    """Insert a step-0 free dim of size n at axis_pos of an AP."""
    new = list(ap.ap)
    new.insert(axis_pos, [0, n])
    return bass.AP(tensor=ap.tensor, offset=ap.offset, ap=new)


def _bcast_part(ap, n):
    """Partition-broadcast AP (step-0 partition dim) for DMA use."""
    return bass.AP(tensor=ap.tensor, offset=ap.offset, ap=[[0, n]] + list(ap.ap[1:]))


def build(lamb1, has_bias, has_ln, debug=False):
    """Build the single-core SPMD program.

    lamb1: python float (v-residual own-value weight; the residual weight
    lamb2 is folded into the host-prescaled vres input).
    has_bias: combined qkv+dt bias is nonzero -> biasd input present.
    has_ln: any qk-LayerNorm affine param nontrivial -> lnp input present.
    """
    nc = bacc.Bacc("TRN2", target_bir_lowering=False)

    xdT = nc.dram_tensor("xdT", [2 * C, N], BF16, kind="ExternalInput")
    w = nc.dram_tensor("w", [2 * C, 3 * HPC * DH], BF16, kind="ExternalInput")
    # vres / rope tables come host-permuted into SBUF layout (p-major).
    vres = nc.dram_tensor("vres", [128, NT * HPC * DH], BF16, kind="ExternalInput")
    ropec = nc.dram_tensor("ropec", [128, NT * DH], BF16, kind="ExternalInput")
    ropes = nc.dram_tensor("ropes", [128, NT * DH], BF16, kind="ExternalInput")
    wproj = nc.dram_tensor("wproj", [HPC * DH, C], BF16, kind="ExternalInput")
    biasd = None
    if has_bias:
        biasd = nc.dram_tensor("biasd", [1, 3 * HPC * DH], F32, kind="ExternalInput")
    lnpd = None
    if has_ln:
        lnpd = nc.dram_tensor("lnp", [4, DH], BF16, kind="ExternalInput")
    out = nc.dram_tensor("out", [N, C], BF16, kind="ExternalOutput")
    dbg = {}
    if debug:
        for nm, shp, dt in [
                ("d_qr", [N, HPC * DH], BF16), ("d_kr", [N, HPC * DH], BF16),
                ("d_kT", [128, N], BF16), ("d_qT", [128, N], BF16),
                ("d_v", [N, HPC * (DH + 1)], BF16),
                ("d_ex", [128, 1024], BF16), ("d_av", [DH + 1, 512], F32),
                ("d_rcp", [1, 512], F32), ("d_rep", [DH, 512], F32),
                ("d_outT", [128, N], BF16)]:
            dbg[nm] = nc.dram_tensor(nm, shp, dt, kind="ExternalOutput")

    with tile.TileContext(nc) as tc:
        with (
            tc.tile_pool(name="const", bufs=1) as constp,
            tc.tile_pool(name="longp", bufs=1) as longp,
        ):
            ident = constp.tile([128, 128], BF16)
            make_identity(nc, ident)
            eps_t = constp.tile([128, 1], F32)
            nc.vector.memset(eps_t, EPS)
            wsrc = constp.tile([128, 128], BF16)
            nc.vector.memset(wsrc, 0.5)
            ones_r = constp.tile([1, DH], F32)
            nc.vector.memset(ones_r, 1.0)

            bias_sb = None
            if biasd is not None:
                bias_sb = constp.tile([128, 3 * HPC * DH], F32)
                nc.scalar.dma_start(out=bias_sb, in_=_bcast_part(biasd[:, :], 128))
            ln_sb = None
            if lnpd is not None:
                ln_sb = constp.tile([128, 4, DH], BF16)
                nc.scalar.dma_start(out=ln_sb, in_=_bcast_part(lnpd[:, :], 128))

            # big persistent SBUF tensors
            xdT_sb = longp.tile([128, KC, N], BF16)
            w_sb = longp.tile([128, KC, 2 * HPC * DH], BF16)
            wv_sb = longp.tile([128, KC, HPC * DH], BF16)
            rpc_sb = longp.tile([128, NT, DH], BF16)
            rps_sb = longp.tile([128, NT, DH], BF16)
            vres_sb = longp.tile([128, NT, HPC, DH], BF16)
            v_sb = longp.tile([128, NT, HPC, DH + 1], BF16)
            qT_sb = longp.tile([128, HPC // 2, N], BF16)
            kT_sb = longp.tile([128, HPC // 2, N], BF16)
            outT_sb = longp.tile([128, HPC // 2, N], BF16)
            wproj_sb = longp.tile([128, 4, C], BF16)

            # ---- input DMAs. Critical order: the tiny rope tables and the
            # first xdT / w(q+k cols) chunks lead; v-projection columns,
            # vres and wproj stream later (first needed mid-phase-A).
            # sync queue: xdT + rope; gpsimd queue: w. ~5MB each.
            nc.sync.dma_start(out=xdT_sb[:, 0, 0:256], in_=xdT[0:128, 0:256])
            nc.gpsimd.dma_start(out=w_sb[:, 0, 0:1024], in_=w[0:128, 0:1024])
            nc.scalar.dma_start(out=rpc_sb,
                                in_=ropec[:, :].rearrange("p (t d) -> p t d", t=NT))
            nc.scalar.dma_start(out=rps_sb,
                                in_=ropes[:, :].rearrange("p (t d) -> p t d", t=NT))
            nc.sync.dma_start(out=xdT_sb[:, 0, 256:N], in_=xdT[0:128, 256:N])
            # first chunks alternate queues so the DMA-paced q-bulk start
            # gets both tensors' early chunks at ~2x cadence
            for kc in range(1, 5):
                qa, qb = ((nc.sync, nc.gpsimd) if kc % 2 == 1
                          else (nc.gpsimd, nc.sync))
                qa.dma_start(out=xdT_sb[:, kc, :],
                             in_=xdT[kc * 128:(kc + 1) * 128, :])
                qb.dma_start(out=w_sb[:, kc, 0:1024],
                             in_=w[kc * 128:(kc + 1) * 128, 0:1024])
            for kc in range(5, KC):
                nc.sync.dma_start(out=xdT_sb[:, kc, :],
                                  in_=xdT[kc * 128:(kc + 1) * 128, :])
                nc.gpsimd.dma_start(out=w_sb[:, kc, 0:1024],
                                    in_=w[kc * 128:(kc + 1) * 128, 0:1024])
            # late tensors: v columns of w, vres, wproj
            nc.scalar.dma_start(
                out=vres_sb,
                in_=vres[:, :].rearrange("p (t h d) -> p t h d", t=NT, h=HPC))
            for kc in range(KC):
                nc.gpsimd.dma_start(out=wv_sb[:, kc, :],
                                    in_=w[kc * 128:(kc + 1) * 128, 1024:1536])
            for cc in range(4):
                (nc.scalar if cc % 2 == 0 else nc.gpsimd).dma_start(
                    out=wproj_sb[:, cc, :], in_=wproj[cc * 128:(cc + 1) * 128, :])
            # prewarm the GpSimd custom-op library (first partition_broadcast
            # otherwise pays a ~7.7us ucode load right when the first softmax
            # finale needs it). The src reads a wproj_sb cell so the scheduler
            # cannot hoist it ahead of the input DMA triggers on the gpsimd
            # queue — the ucode load blocks that queue for its duration.
            gwarm = constp.tile([DH, 1], BF16)
            nc.gpsimd.partition_broadcast(gwarm[:, :], wproj_sb[0:1, 3, 0:1],
                                          channels=DH)

            # ones column of v (denominator row of the av matmul)
            nc.vector.memset(v_sb[:, :, :, DH:DH + 1], 1.0)

            # ------------- phase A: fused qkv+dt projection, LN, rope ------
            # q runs kc-outer (tracks DMA chunk arrival); k and v run t-outer
            # with inline postprocessing so PSUM tiles free progressively.
            # Each projection tile is first copied to bf16 SBUF by ScalarE —
            # that copy is the tile's only PSUM reader, so the bank frees
            # ~1us after the matmuls finish, and the LN/rope math runs on
            # fast packed-bf16 SBUF DVE ops. The q/k weights are host-
            # centered, so LN needs no mean subtraction here.
            with (
                tc.tile_pool(name="qkp", bufs=1) as qkp,
                tc.tile_pool(name="psA", bufs=8, space="PSUM") as psA,
                tc.tile_pool(name="scrA", bufs=3) as scrA,
                tc.tile_pool(name="stat", bufs=4) as stat,
            ):
                qr_sb = qkp.tile([128, NT, HPC, DH], BF16)
                kr_sb = qkp.tile([128, NT, HPC, DH], BF16)

                # HAM warmup: keep the PE busy during the DMA-wait head so
                # the clock gate is at 8/8 when the real matmuls arrive.
                # HAM warmup so the real matmuls (first DMA-gated, ~9-10us
                # in) start at the full 2.4 GHz clock. Fed by a memset tile
                # (no make_identity dependency) so it starts ~6us in.
                warm = psA.tile([128, 512], F32, name="warm", tag="pp")
                for _ in range(24):
                    nc.tensor.matmul(warm[:, 0:128], wsrc[:], wsrc[:],
                                     start=True, stop=True)

                def post_qk1(ps, t, ob, on_dve=False):
                    """PSUM-freeing copy only — emitted inline with the
                    projection matmuls so banks release at copy pace."""
                    ps3 = ps.rearrange("p (h d) -> p h d", h=HPC)
                    if bias_sb is not None:
                        nc.vector.tensor_add(
                            ps[:], ps[:], bias_sb[:, ob * 512:(ob + 1) * 512])
                    xb = scrA.tile([128, HPC, DH], BF16, tag="xb", bufs=17)
                    if on_dve:
                        nc.vector.tensor_copy(xb[:], ps3)
                    else:
                        nc.scalar.activation(out=xb[:], in_=ps3, func=AF.Copy)
                    return xb

                def post_qk2(xb, t, ob):
                    """LN rstd + rope chain (DVE), deferred past the copies."""
                    sqb = scrA.tile([128, HPC, DH], BF16, tag="sqb")
                    nc.vector.tensor_mul(sqb[:], xb[:], xb[:])
                    red_q = stat.tile([128, HPC], F32, tag="red_q")
                    nc.vector.reduce_sum(out=red_q[:], in_=sqb[:], axis=AX)
                    # rstd = 1/sqrt(sum(q^2)/DH + eps); Sqrt keeps one ACT
                    # table set in phase A, DVE reciprocal leaves Exp's
                    # table untouched for the attention phase.
                    rstd = stat.tile([128, HPC], F32, tag="rstd")
                    nc.scalar.activation(out=rstd[:], in_=red_q[:], func=AF.Sqrt,
                                         scale=1.0 / DH, bias=eps_t[:])
                    nc.vector.reciprocal(rstd[:], rstd[:])
                    xr = scrA.tile([128, HPC, DH], BF16, tag="xr")
                    nc.vector.tensor_tensor(
                        out=xr[:], in0=xb[:], in1=_bcast_free(rstd[:], DH, 2)[:],
                        op=ALU.mult)
                    if ln_sb is not None:
                        gi, bi = (0, 1) if ob == 0 else (2, 3)
                        nc.vector.tensor_tensor(
                            out=xr[:], in0=xr[:],
                            in1=_bcast_free(ln_sb[:, gi, :], HPC, 1)[:],
                            op=ALU.mult)
                        nc.vector.tensor_tensor(
                            out=xr[:], in0=xr[:],
                            in1=_bcast_free(ln_sb[:, bi, :], HPC, 1)[:],
                            op=ALU.add)
                    # rope: dst = xr*cos + rot_half(xr)*sin'
                    dst = (qr_sb if ob == 0 else kr_sb)
                    rpc = _bcast_free(rpc_sb[:, t], HPC, 1)
                    rps_lo = _bcast_free(rps_sb[:, t, 0:HD], HPC, 1)
                    rps_hi = _bcast_free(rps_sb[:, t, HD:DH], HPC, 1)
                    tc_ = scrA.tile([128, HPC, DH], BF16, tag="tc")
                    nc.vector.tensor_tensor(out=tc_[:], in0=xr[:],
                                            in1=rpc[:], op=ALU.mult)
                    tm = scrA.tile([128, HPC, DH], BF16, tag="tm")
                    nc.vector.tensor_tensor(out=tm[:, :, 0:HD],
                                            in0=xr[:, :, HD:DH],
                                            in1=rps_lo[:], op=ALU.mult)
                    nc.vector.tensor_tensor(out=tm[:, :, HD:DH],
                                            in0=xr[:, :, 0:HD],
                                            in1=rps_hi[:], op=ALU.mult)
                    nc.vector.tensor_tensor(out=dst[:, t], in0=tc_[:],
                                            in1=tm[:], op=ALU.add)

                def post_v(ps, t, on_dve):
                    ps3 = ps.rearrange("p (h d) -> p h d", h=HPC)
                    if bias_sb is not None:
                        nc.vector.tensor_add(
                            ps[:], ps[:], bias_sb[:, 1024:1536])
                    if on_dve:
                        # attention-phase tiles: keep ScalarE free for exps
                        nc.vector.tensor_scalar_mul(
                            v_sb[:, t, :, 0:DH], in0=ps3, scalar1=float(lamb1))
                    else:
                        nc.scalar.activation(out=v_sb[:, t, :, 0:DH], in_=ps3,
                                             func=AF.Copy, scale=float(lamb1))
                    nc.vector.tensor_tensor(out=v_sb[:, t, :, 0:DH],
                                            in0=v_sb[:, t, :, 0:DH],
                                            in1=vres_sb[:, t], op=ALU.add)

                def proj_mms(ps, ob, t, kcs):
                    for kc in kcs:
                        wsl = (wv_sb[:, kc, :] if ob == 2
                               else w_sb[:, kc, ob * 512:(ob + 1) * 512])
                        nc.tensor.matmul(
                            ps[:],
                            xdT_sb[:, kc, t * 128:(t + 1) * 128],
                            wsl,
                            start=(kc == 0), stop=(kc == KC - 1))

                def proj_tile(ob, t, pool, tag):
                    ps = pool.tile([128, 512], F32, name=f"pt{ob}_{t}",
                                   tag=tag)
                    proj_mms(ps, ob, t, range(KC))
                    return ps

                def transpose_ob(src, dstT):
                    for j in range(HPC // 2):
                        tp = psA.tile([128, 512], F32, tag="pp")
                        tpb = tp.bitcast(BF16)
                        for t in range(NT):
                            nc.tensor.transpose(
                                tpb[:, t * 128:(t + 1) * 128],
                                src[:, t, 2 * j:2 * j + 2, :]
                                   .rearrange("p h d -> p (h d)"),
                                ident[:])
                        if dstT is qT_sb:
                            nc.scalar.activation(out=dstT[:, j, :], in_=tpb[:],
                                                 func=AF.Copy)
                        else:
                            nc.vector.tensor_copy(dstT[:, j, :], tpb[:])

                # q: bulk kc-outer (tracks DMA chunk arrival), then a
                # t-outer tail with inline posts so the DVE postprocess
                # stream is spread instead of bursting 8 chains at once.
                KS = KC
                ps_tiles = [psA.tile([128, 512], F32, name=f"pp{_t}",
                                     tag="pp")
                            for _t in range(NT)]
                for kc in range(KS):
                    for t in range(NT):
                        nc.tensor.matmul(
                            ps_tiles[t][:],
                            xdT_sb[:, kc, t * 128:(t + 1) * 128],
                            w_sb[:, kc, 0:512],
                            start=(kc == 0), stop=False)
                q_xb = []
                for t in range(NT):
                    for kc in range(KS, KC):
                        nc.tensor.matmul(
                            ps_tiles[t][:],
                            xdT_sb[:, kc, t * 128:(t + 1) * 128],
                            w_sb[:, kc, 0:512],
                            start=False, stop=(kc == KC - 1))
                    # DVE is idle here (part2 chains deferred), so alternate
                    # the copies across engines for 2x bank-release pace
                    q_xb.append(post_qk1(ps_tiles[t], t, 0,
                                         on_dve=(t % 2 == 1)))
                for t in range(NT):
                    post_qk2(q_xb[t], t, 0)
                # k (copies on ScalarE — DVE is draining the q chains),
                # then both transposes, then the first half of v
                k_xb = []
                for t in range(NT):
                    k_xb.append(post_qk1(proj_tile(1, t, psA, "pp"), t, 1))
                for t in range(NT):
                    post_qk2(k_xb[t], t, 1)
                # v tiles before the transposes: independent PE work covers
                # the DVE draining the k postprocess chains, so the
                # transposes (which need every k tile's rope done) run
                # gap-free right before attention consumes them.
                transpose_ob(qr_sb, qT_sb)
                for t in range(6):
                    post_v(proj_tile(2, t, psA, "pp"), t, on_dve=False)
                transpose_ob(kr_sb, kT_sb)
                # preload the Exp ACT table (no more Sqrts follow) so the
                # ~2.7us table switch overlaps the v tiles, not the first
                # attention exp.
                nc.scalar.activation(out=eps_t[:], in_=eps_t[:], func=AF.Exp)
                if debug:
                    rr2 = "(t p) (h d) -> p t h d"
                    nc.sync.dma_start(
                        out=dbg["d_qr"][:, :].rearrange(rr2, p=128, h=HPC),
                        in_=qr_sb)
                    nc.sync.dma_start(
                        out=dbg["d_kr"][:, :].rearrange(rr2, p=128, h=HPC),
                        in_=kr_sb)
                    nc.sync.dma_start(out=dbg["d_qT"][:, :], in_=qT_sb[:, 0, :])
                    nc.sync.dma_start(out=dbg["d_kT"][:, :], in_=kT_sb[:, 0, :])

            # ------------- attention + interleaved fillers ------------------
            # (j, qh) pair-major iteration covering both 64-dim head-halves.
            # The two halves' score matmuls are interleaved at tile_position
            # rows 0/64 so the PE row-tiles them concurrently. PE filler
            # between attention chunks: first the deferred second half of
            # the v projection (tiles 4-7, emitted in 8-matmul halves), then
            # out-projection tiles once a query half's finales land.
            with (
                tc.tile_pool(name="psS", bufs=2, space="PSUM") as psS,
                tc.tile_pool(name="psV", bufs=3, space="PSUM") as psV,
                tc.tile_pool(name="psP", bufs=1, space="PSUM") as psP,
                tc.tile_pool(name="expp", bufs=8) as expp,
                tc.tile_pool(name="nrm", bufs=2) as nrm,
                tc.tile_pool(name="outp", bufs=2) as outp,
            ):
                pairs = [(j, qh) for qh in range(2) for j in range(HPC // 2)]
                NSTEP = NT // 2  # kk steps per pair (2 key chunks per half)
                av_t = {}
                scale = 1.0 / float(np.sqrt(DH))

                def emit_sc_pair(pi, kc):
                    # one chunk-pair per step: a single [128,2,512] psum tile
                    # holds BOTH head-halves' score chunks, so their WAR
                    # clears atomically — the scheduler keeps the A/B
                    # interleave and the PE row-tiles the two 64-contraction
                    # matmuls concurrently. bufs=2 gives one step of slack
                    # between the sc matmuls and the exp of the prior step.
                    j, qh = pairs[pi]
                    sc = psS.tile([128, 2, 512], F32, tag="sc", bufs=2)
                    qs = slice(qh * 512, (qh + 1) * 512)
                    ks = slice(kc * 128, (kc + 1) * 128)
                    nc.tensor.matmul(
                        sc[:, 0, :], kT_sb[0:DH, j, ks], qT_sb[0:DH, j, qs],
                        start=True, stop=True, tile_position=(0, 0))
                    nc.tensor.matmul(
                        sc[:, 1, :], kT_sb[DH:128, j, ks],
                        qT_sb[DH:128, j, qs],
                        start=True, stop=True, tile_position=(DH, 0))
                    ex = expp.tile([128, 2, 512], BF16, tag="ex", bufs=8)
                    if pi == len(pairs) - 1:
                        # final pair: per-half exps so the drain chain
                        # (exp -> av -> finale -> out-proj) starts sooner
                        for i in range(2):
                            nc.scalar.activation(out=ex[:, i, :],
                                                 in_=sc[:, i, :],
                                                 func=AF.Exp, scale=scale)
                    else:
                        nc.scalar.activation(out=ex[:], in_=sc[:],
                                             func=AF.Exp, scale=scale)
                    if debug and pi == 0 and kc == 0:
                        nc.sync.dma_start(
                            out=dbg["d_ex"][:, 0:1024].rearrange(
                                "p (a b) -> p a b", a=2),
                            in_=ex[:])
                    return ex

                def emit_av_pair(pi, kc, ex):
                    j, qh = pairs[pi]
                    if kc == 0:
                        av_t[(pi, 0)] = psV.tile([DH + 1, 512], F32,
                                                 name=f"av{pi}_0", tag="av")
                        av_t[(pi, 1)] = psV.tile([DH + 1, 512], F32,
                                                 name=f"av{pi}_1", tag="av")
                    nc.tensor.matmul(
                        av_t[(pi, 0)][:], v_sb[:, kc, 2 * j, :],
                        ex[:, 0, :], start=(kc == 0), stop=(kc == NT - 1))
                    nc.tensor.matmul(
                        av_t[(pi, 1)][:], v_sb[:, kc, 2 * j + 1, :],
                        ex[:, 1, :], start=(kc == 0), stop=(kc == NT - 1))

                def emit_finale(pi, hh):
                    j, qh = pairs[pi]
                    ro = 64 * hh
                    av = av_t.pop((pi, hh))
                    rep = nrm.tile([DH, 2, 512], F32, tag="rep")
                    # sums row (psum partition 64) -> partition 0 SBUF on
                    # DVE (ScalarE is saturated with exps), then gpsimd-
                    # broadcast to 64 partitions, then approx-recip there
                    # (the custom DVE op misbehaves at base >= 64).
                    sums = nrm.tile([1, 512], F32, tag="sums")
                    if pi == len(pairs) - 1:
                        # ScalarE is free once the last exps retire; taking
                        # the sums copy there shortens the serialized DVE
                        # chain that gates the final out-proj units
                        nc.scalar.activation(out=sums[:], in_=av[DH:DH + 1, :],
                                             func=AF.Copy)
                    else:
                        nc.vector.tensor_copy(sums[:], av[DH:DH + 1, :])
                    # raw-av copy frees the psum bank ~1.4us after the last
                    # av matmul (instead of after the whole 3us normalize
                    # chain) so the next pair's av allocation never stalls.
                    avr = nrm.tile([DH, 512], F32, tag="avr", bufs=3)
                    nc.vector.tensor_copy(avr[:], av[0:DH, :])
                    nc.gpsimd.partition_broadcast(
                        rep[:, 0, :], sums[:], channels=DH)
                    nc.vector.reciprocal_approx_fast(
                        out=rep[:, 1, :], in_=rep[:, 0, :])
                    nc.vector.tensor_tensor(
                        out=outT_sb[ro:ro + DH, j, qh * 512:(qh + 1) * 512],
                        in0=avr[:], in1=rep[:, 1, :], op=ALU.mult)
                    if debug and pi == 0 and hh == 0:
                        avc = nrm.tile([DH + 1, 512], F32, tag="avc")
                        nc.vector.tensor_copy(avc[:], av[:])
                        nc.sync.dma_start(out=dbg["d_av"][:, :], in_=avc)
                        nc.sync.dma_start(out=dbg["d_rcp"][:, :],
                                          in_=rep[0:1, 1, :])
                        nc.sync.dma_start(out=dbg["d_rep"][:, :],
                                          in_=rep[:, 1, :])

                # qh0 units (t 0..3) run fully as attention fillers. qh1
                # units (t 4..7) run in two stages: cc 0..2 partial-summed
                # to SBUF f32 during the last pair (j0..2 finales are in),
                # then one cc=3 matmul + DVE add after the last finale.
                proj_units = [(t, oh) for t in range(NT) for oh in range(2)]
                stg_t = {}
                prt = {}
                state = {"emitted": 0, "finales": 0, "vdef": 0, "partial": 8}

                def unit_mms(pp, t, oh, ccs, start_cc, stop_cc):
                    for cc in ccs:
                        nc.tensor.matmul(
                            pp[:],
                            outT_sb[:, cc, t * 128:(t + 1) * 128],
                            wproj_sb[:, cc, oh * 512:(oh + 1) * 512],
                            start=(cc == start_cc), stop=(cc == stop_cc))

                def emit_proj_unit():
                    t, oh = proj_units[state["emitted"]]
                    state["emitted"] += 1
                    if oh == 0:
                        stg_t[t] = outp.tile([128, C], BF16, name=f"stg{t}",
                                             tag="stg")
                    pp = psP.tile([128, 512], F32, tag="pp2")
                    unit_mms(pp, t, oh, range(4), 0, 3)
                    nc.vector.tensor_copy(
                        stg_t[t][:, oh * 512:(oh + 1) * 512], pp[:])
                    if oh == 1:
                        nc.sync.dma_start(out=out[t * 128:(t + 1) * 128, :],
                                          in_=stg_t.pop(t))

                def emit_partial_unit():
                    # t=4,5 units: cc0..2 partial -> SBUF f32 (final = DVE add)
                    t, oh = proj_units[state["partial"]]
                    state["partial"] += 1
                    pp = psP.tile([128, 512], F32, tag="pp2")
                    unit_mms(pp, t, oh, range(3), 0, 2)
                    prt[(t, oh)] = outp.tile([128, 512], F32,
                                             name=f"prt{t}_{oh}", tag="prt",
                                             bufs=4)
                    nc.vector.tensor_copy(prt[(t, oh)][:], pp[:])

                prt_ps = {}

                def emit_psum_partials():
                    # t=6,7 units: cc0..2 stay resident in psS tiles (their
                    # sc traffic is over); final = one accumulating matmul +
                    # a ScalarE copy, so the tail splits across DVE + ScalarE
                    for t in (6, 7):
                        pt = psS.tile([128, 2, 512], F32, name=f"prtps{t}",
                                      tag="sc", bufs=2)
                        for oh in range(2):
                            unit_mms(pt[:, oh, :], t, oh, range(3), 0, -1)
                        prt_ps[t] = pt

                def emit_final_unit(t, oh):
                    if oh == 0:
                        stg_t[t] = outp.tile([128, C], BF16, name=f"stg{t}",
                                             tag="stg")
                    if t in prt_ps:
                        pt = prt_ps[t]
                        unit_mms(pt[:, oh, :], t, oh, [3], -1, 3)
                        nc.scalar.activation(
                            out=stg_t[t][:, oh * 512:(oh + 1) * 512],
                            in_=pt[:, oh, :], func=AF.Copy)
                    else:
                        # the av banks are all retired by now — rotating the
                        # final-unit psums through psV avoids serializing on
                        # the single psP bank
                        pp = psV.tile([128, 512], F32, tag="av",
                                      name=f"fu{t}_{oh}")
                        unit_mms(pp, t, oh, [3], 3, 3)
                        nc.vector.tensor_tensor(
                            out=stg_t[t][:, oh * 512:(oh + 1) * 512],
                            in0=pp[:], in1=prt.pop((t, oh))[:], op=ALU.add)
                    if oh == 1:
                        nc.sync.dma_start(out=out[t * 128:(t + 1) * 128, :],
                                          in_=stg_t.pop(t))

                # deferred v tiles 4-7 spread over attention steps 0-5;
                # deferred v tiles 4-7 spread as half-tile fillers over the
                # first chunk-steps; tile T's second half lands at step
                # 2(T-4)+1, well before its first av reader (kc=T) is
                # emitted at step T+6 with the lookahead of 6.
                vplan = [(6, 0), (6, 1), (7, 0), (7, 1)]
                vps = {}

                def emit_filler():
                    if state["vdef"] < len(vplan):
                        t, half = vplan[state["vdef"]]
                        state["vdef"] += 1
                        if half == 0:
                            vps[t] = psP.tile([128, 512], F32,
                                              name=f"vt{t}", tag="pp2")
                            proj_mms(vps[t], 2, t, range(0, KC // 2))
                        else:
                            proj_mms(vps[t], 2, t, range(KC // 2, KC))
                            post_v(vps.pop(t), t, on_dve=True)
                        return
                    if state["finales"] >= 8 and state["emitted"] < 8:
                        emit_proj_unit()
                        return
                    if state["finales"] >= 14 and state["partial"] < 12:
                        emit_partial_unit()
                        if state["partial"] < 12:
                            emit_partial_unit()

                # pipeline: sc/exp run 6 chunk-steps ahead of av; finales
                # (all DVE/GpSimd) are emitted as soon as the last av lands.
                steps = [(pi, kc) for pi in range(len(pairs))
                         for kc in range(NT)]
                exq = []      # (pi, kc, ex) awaiting av emission
                fill_tick = 0
                for (pi, kc) in steps:
                    # last pair: drop the av lookahead to 1 so the drain
                    # chain (av -> finale -> out-proj tail) starts sooner
                    look = 1 if pi == len(pairs) - 1 else 6
                    while len(exq) >= look:
                        api, akc, aex = exq.pop(0)
                        emit_av_pair(api, akc, aex)
                        if akc == NT - 1:
                            emit_finale(api, 0)
                            emit_finale(api, 1)
                            state["finales"] += 2
                    exq.append((pi, kc, emit_sc_pair(pi, kc)))
                    # fillers are ~1.7-1us of PE work; one per two chunk-
                    # steps keeps the PE just above the exp pace
                    fill_tick += 1
                    if state["vdef"] < len(vplan) or fill_tick % 2 == 0:
                        emit_filler()
                for (api, akc, aex) in exq:
                    emit_av_pair(api, akc, aex)
                    if akc == NT - 1:
                        emit_finale(api, 0)
                        emit_finale(api, 1)
                        state["finales"] += 2
                if debug:
                    nc.sync.dma_start(out=dbg["d_outT"][:, :],
                                      in_=outT_sb[:, 0, :])
                    nc.sync.dma_start(
                        out=dbg["d_v"][:, :].rearrange(
                            "(t p) (h d) -> p t h d", p=128, h=HPC),
                        in_=v_sb)
                while state["emitted"] < 8:
                    emit_proj_unit()
                while state["partial"] < 12:
                    emit_partial_unit()
                emit_psum_partials()
                # interleave DVE-add units (t4,5) with ScalarE-copy units
                # (t6,7) so the tail splits across both engines
                for t in (4, 6, 5, 7):
                    emit_final_unit(t, 0)
                    emit_final_unit(t, 1)

    nc.finalize()
    return nc


_CACHE = {}
_LAST_RES = None


def _bf16(a):
    return np.ascontiguousarray(a.astype(ml_dtypes.bfloat16))


def kernel(x, rope, delta_t_emb, v_residual_v1, Wqkv, bqkv, Wdt, bdt,
           qn_g, qn_b, kn_g, kn_b, lamb1, lamb2, Wproj, bproj):
    x = np.asarray(x, np.float32)
    rope = np.ascontiguousarray(np.asarray(rope, np.float32))
    delta_t_emb = np.asarray(delta_t_emb, np.float32)
    v_residual_v1 = np.asarray(v_residual_v1, np.float32)
    Wqkv = np.asarray(Wqkv, np.float32)
    Wdt = np.asarray(Wdt, np.float32)
    Wproj = np.asarray(Wproj, np.float32)
    bias = np.asarray(bqkv, np.float32) + np.asarray(bdt, np.float32)
    l1 = float(np.asarray(lamb1)); l2 = float(np.asarray(lamb2))
    qn_g = np.asarray(qn_g, np.float32); qn_b = np.asarray(qn_b, np.float32)
    kn_g = np.asarray(kn_g, np.float32); kn_b = np.asarray(kn_b, np.float32)

    has_bias = bool(np.any(bias))
    has_ln = not (np.all(qn_g == 1.0) and np.all(qn_b == 0.0)
                  and np.all(kn_g == 1.0) and np.all(kn_b == 0.0))

    dbgf = bool(int(os.environ.get("KERNEL_DEBUG", "0")))
    key = (l1, has_bias, has_ln, dbgf)
    if key not in _CACHE:
        _CACHE[key] = build(l1, has_bias, has_ln, debug=dbgf)
    nc = _CACHE[key]

    # host-prepared rope tables in SBUF layout [p, t*DH]:
    # cos table and sign-folded sin table (rotate_half absorbed:
    # out = x*cos + rot(x)*sin' with sin' = [-sin_lo || sin_hi]).
    sin = rope[:, 0:DH]; cos = rope[:, DH:2 * DH]
    sinp = np.concatenate([-sin[:, 0:HD], sin[:, HD:DH]], axis=1)

    def _ptile(a):  # [N, DH] -> [128, NT*DH] with n = t*128 + p
        return _bf16(a.reshape(NT, 128, DH).transpose(1, 0, 2).reshape(128, -1))

    cos_p = _ptile(cos)
    sin_p = _ptile(sinp)

    in_maps = []
    for c in range(8):
        b = c // 2
        g = c % 2
        rsl = slice(g * 512, (g + 1) * 512)
        w_core = np.concatenate([
            np.concatenate([Wqkv[rsl], Wqkv[C:][rsl], Wqkv[2 * C:][rsl]], 0).T,
            np.concatenate([Wdt[rsl], Wdt[C:][rsl], Wdt[2 * C:][rsl]], 0).T,
        ], axis=0)
        w_core = np.ascontiguousarray(w_core)
        bc = np.concatenate([bias[rsl], bias[C:][rsl], bias[2 * C:][rsl]])
        bc = bc.astype(np.float32).copy()
        # fold LN mean-centering into the q/k weight+bias head blocks
        # (exact: (x@W + b)@C = x@(W@C) + b@C with C = I - J/64)
        for ob in range(2):
            for h in range(HPC):
                sl = slice(ob * 512 + h * DH, ob * 512 + (h + 1) * DH)
                w_core[:, sl] -= w_core[:, sl].mean(axis=1, keepdims=True)
                bc[sl] -= bc[sl].mean()
        # vres in SBUF layout [p, t, h, d] flattened
        vr = (l2 * v_residual_v1[b, g * 8:(g + 1) * 8]).transpose(1, 0, 2)
        vr = vr.reshape(NT, 128, HPC, DH).transpose(1, 0, 2, 3).reshape(128, -1)
        # rotate each core's contraction-chunk order (sum order is free):
        # the 8 SPMD cores otherwise request the SAME chunk simultaneously
        # and serialize on HBM during the critical early stream
        rot = (2 * c) % KC
        perm = [(k + rot) % KC for k in range(KC)]
        xdT_full = np.concatenate([x[b].T, delta_t_emb[b].T], 0)
        xdT_rot = np.concatenate([xdT_full[128 * k:128 * (k + 1)]
                                  for k in perm], 0)
        w_rot = np.concatenate([w_core[128 * k:128 * (k + 1)]
                                for k in perm], 0)
        m = {
            "xdT": _bf16(xdT_rot),
            "w": _bf16(w_rot),
            "vres": _bf16(vr),
            "ropec": cos_p,
            "ropes": sin_p,
            "wproj": _bf16(Wproj[:, rsl].T),
        }
        if has_bias:
            m["biasd"] = np.ascontiguousarray(bc[None, :].astype(np.float32))
        if has_ln:
            m["lnp"] = _bf16(np.stack([qn_g, qn_b, kn_g, kn_b], 0))
        in_maps.append(m)

    trace = bool(int(os.environ.get("KERNEL_TRACE", "0")))
    res = run_bass_kernel_spmd(nc, in_maps, core_ids=list(range(8)), trace=trace)
    global _LAST_RES
    _LAST_RES = res
    if trace and res.exec_time_ns is not None:
        print(f"HW exec time: {res.exec_time_ns} ns")
        kernel.last_exec_time_ns = res.exec_time_ns
        kernel.last_results = res

    out = np.empty((B, N, C), np.float32)
    for b in range(B):
        out[b] = (res.results[2 * b]["out"].astype(np.float32)
                  + res.results[2 * b + 1]["out"].astype(np.float32))
    bproj = np.asarray(bproj, np.float32)
    if np.any(bproj):
        out += bproj[None, None, :]
    return out


# revision 70
# speedup vs baseline: 1.1620x; 1.0016x over previous
"""Trainium2 Bass kernel for nn_AttentionBlock (B=4, N=1024, C=1024, H=16).

Sharding: 8 cores = 4 batches x 2 head-groups (8 heads each). Each core
computes its batch's tokens for its 8 heads end-to-end (fused qkv+delta
projection, qk-LayerNorm, RoPE, softmax attention with value-residual mix,
and a partial output projection over its head columns). The host sums the
two partial projections per batch.

Measured ~202us on 8 trn2 cores (baseline lineage: 452us f32r ->
253us bf16 -> 202us this version). Key optimizations:
- all matmul operands bf16; LN mean-centering folded into the q/k weight
  columns on the host (exact: (x@W)@C = x@(W@C) with C = I - J/64), so the
  in-kernel LN is just rstd scaling (fp8 DoubleRow was tried and rejected:
  attention-weight/v quantization error does NOT average down — it scales
  with the same sqrt(sum a^2) factor as the signal -> ~5e-2 rel err);
- rope/vres tables are host-permuted to the exact SBUF layout so their
  DMAs are contiguous (128 descriptors, not 8192) and ride the idle
  scalar queue; w streams q+k columns before v columns (separate wv tile
  to avoid false chunk-row deps), early chunks alternate queues; each
  core's contraction chunks are host-rotated by 2*core_id so the 8 SPMD
  cores don't request the same HBM region simultaneously;
- score matmuls for the two 64-dim head-halves live in ONE [128,2,512]
  psum tile (their WAR clears atomically, so the scheduler keeps the A/B
  interleave) and run concurrently via tile_position row tiling;
- chunk-grained attention pipeline: per step one sc pair + one 1024-wide
  exp, av matmuls trail by 6 steps, psS bufs=2 gives the exp stream one
  step of slack; psV bufs=3 plus an early raw-av copy (normalize multiply
  reads the copy) so av psum banks free ~1.4us after their last matmul;
- warmup matmuls during the DMA-wait head keep the PE HAM clock at 8/8;
  the GpSimd broadcast ucode library is prewarmed behind the input DMA
  triggers (first use otherwise costs ~7.7us on the gpsimd queue);
- softmax normalize: ones-column denominator row -> GpSimd broadcast ->
  DVE approx-reciprocal -> one multiply; the Exp ACT table is preloaded
  during phase A so no table switch gates the first attention exp;
- tail: qh1 output-projection units are partial-summed (cc 0..2) during
  the last pair — two via SBUF f32 partials (DVE add) and two resident in
  retired score psum banks (ScalarE copy) — so only one matmul plus one
  copy/add per unit remains after the final softmax finale.
"""
import os
import sys

sys.path.insert(0, "/opt/trn_rl_repo")

import numpy as np
import ml_dtypes

import concourse.bass as bass
import concourse.bacc as bacc
import concourse.tile as tile
from concourse import mybir
from concourse.bass_utils import run_bass_kernel_spmd
from concourse.masks import make_identity

F32 = mybir.dt.float32
BF16 = mybir.dt.bfloat16

B, N, C, H = 4, 1024, 1024, 16
DH = C // H            # 64
HD = DH // 2           # 32
HPC = 8                # heads per core
NT = N // 128          # 8 token tiles
KC = (2 * C) // 128    # 16 contraction chunks for fused qkv+dt
EPS = 1e-5
AX = mybir.AxisListType.X
ALU = mybir.AluOpType
AF = mybir.ActivationFunctionType


def _bcast_free(ap, n, axis_pos=1):
    """Insert a step-0 free dim of size n at axis_pos of an AP."""
    new = list(ap.ap)
    new.insert(axis_pos, [0, n])
    return bass.AP(tensor=ap.tensor, offset=ap.offset, ap=new)


def _bcast_part(ap, n):
    """Partition-broadcast AP (step-0 partition dim) for DMA use."""
    return bass.AP(tensor=ap.tensor, offset=ap.offset, ap=[[0, n]] + list(ap.ap[1:]))


def build(lamb1, has_bias, has_ln, debug=False):
    """Build the single-core SPMD program.

    lamb1: python float (v-residual own-value weight; the residual weight
    lamb2 is folded into the host-prescaled vres input).
    has_bias: combined qkv+dt bias is nonzero -> biasd input present.
    has_ln: any qk-LayerNorm affine param nontrivial -> lnp input present.
    """
    nc = bacc.Bacc("TRN2", target_bir_lowering=False)

    xdT = nc.dram_tensor("xdT", [2 * C, N], BF16, kind="ExternalInput")
    w = nc.dram_tensor("w", [2 * C, 3 * HPC * DH], BF16, kind="ExternalInput")
    # vres / rope tables come host-permuted into SBUF layout (p-major).
    vres = nc.dram_tensor("vres", [128, NT * HPC * DH], BF16, kind="ExternalInput")
    ropec = nc.dram_tensor("ropec", [128, NT * DH], BF16, kind="ExternalInput")
    ropes = nc.dram_tensor("ropes", [128, NT * DH], BF16, kind="ExternalInput")
    wproj = nc.dram_tensor("wproj", [HPC * DH, C], BF16, kind="ExternalInput")
    biasd = None
    if has_bias:
        biasd = nc.dram_tensor("biasd", [1, 3 * HPC * DH], F32, kind="ExternalInput")
    lnpd = None
    if has_ln:
        lnpd = nc.dram_tensor("lnp", [4, DH], BF16, kind="ExternalInput")
    out = nc.dram_tensor("out", [N, C], BF16, kind="ExternalOutput")
    dbg = {}
    if debug:
        for nm, shp, dt in [
                ("d_qr", [N, HPC * DH], BF16), ("d_kr", [N, HPC * DH], BF16),
                ("d_kT", [128, N], BF16), ("d_qT", [128, N], BF16),
                ("d_v", [N, HPC * (DH + 1)], BF16),
                ("d_ex", [128, 1024], BF16), ("d_av", [DH + 1, 512], F32),
                ("d_rcp", [1, 512], F32), ("d_rep", [DH, 512], F32),
                ("d_outT", [128, N], BF16)]:
            dbg[nm] = nc.dram_tensor(nm, shp, dt, kind="ExternalOutput")

    with tile.TileContext(nc) as tc:
        with (
            tc.tile_pool(name="const", bufs=1) as constp,
            tc.tile_pool(name="longp", bufs=1) as longp,
        ):
            ident = constp.tile([128, 128], BF16)
            make_identity(nc, ident)
            eps_t = constp.tile([128, 1], F32)
            nc.vector.memset(eps_t, EPS)
            wsrc = constp.tile([128, 128], BF16)
            nc.vector.memset(wsrc, 0.5)
            ones_r = constp.tile([1, DH], F32)
            nc.vector.memset(ones_r, 1.0)

            bias_sb = None
            if biasd is not None:
                bias_sb = constp.tile([128, 3 * HPC * DH], F32)
                nc.scalar.dma_start(out=bias_sb, in_=_bcast_part(biasd[:, :], 128))
            ln_sb = None
            if lnpd is not None:
                ln_sb = constp.tile([128, 4, DH], BF16)
                nc.scalar.dma_start(out=ln_sb, in_=_bcast_part(lnpd[:, :], 128))

            # big persistent SBUF tensors
            xdT_sb = longp.tile([128, KC, N], BF16)
            w_sb = longp.tile([128, KC, 2 * HPC * DH], BF16)
            wv_sb = longp.tile([128, KC, HPC * DH], BF16)
            rpc_sb = longp.tile([128, NT, DH], BF16)
            rps_sb = longp.tile([128, NT, DH], BF16)
            vres_sb = longp.tile([128, NT, HPC, DH], BF16)
            v_sb = longp.tile([128, NT, HPC, DH + 1], BF16)
            qT_sb = longp.tile([128, HPC // 2, N], BF16)
            kT_sb = longp.tile([128, HPC // 2, N], BF16)
            outT_sb = longp.tile([128, HPC // 2, N], BF16)
            wproj_sb = longp.tile([128, 4, C], BF16)

            # ---- input DMAs. Critical order: the tiny rope tables and the
            # first xdT / w(q+k cols) chunks lead; v-projection columns,
            # vres and wproj stream later (first needed mid-phase-A).
            # sync queue: xdT + rope; gpsimd queue: w. ~5MB each.
            nc.sync.dma_start(out=xdT_sb[:, 0, 0:256], in_=xdT[0:128, 0:256])
            nc.gpsimd.dma_start(out=w_sb[:, 0, 0:1024], in_=w[0:128, 0:1024])
            nc.scalar.dma_start(out=rpc_sb,
                                in_=ropec[:, :].rearrange("p (t d) -> p t d", t=NT))
            nc.scalar.dma_start(out=rps_sb,
                                in_=ropes[:, :].rearrange("p (t d) -> p t d", t=NT))
            nc.sync.dma_start(out=xdT_sb[:, 0, 256:N], in_=xdT[0:128, 256:N])
            # first chunks alternate queues so the DMA-paced q-bulk start
            # gets both tensors' early chunks at ~2x cadence
            for kc in range(1, 5):
                qa, qb = ((nc.sync, nc.gpsimd) if kc % 2 == 1
                          else (nc.gpsimd, nc.sync))
                qa.dma_start(out=xdT_sb[:, kc, :],
                             in_=xdT[kc * 128:(kc + 1) * 128, :])
                qb.dma_start(out=w_sb[:, kc, 0:1024],
                             in_=w[kc * 128:(kc + 1) * 128, 0:1024])
            for kc in range(5, KC):
                nc.sync.dma_start(out=xdT_sb[:, kc, :],
                                  in_=xdT[kc * 128:(kc + 1) * 128, :])
                nc.gpsimd.dma_start(out=w_sb[:, kc, 0:1024],
                                    in_=w[kc * 128:(kc + 1) * 128, 0:1024])
            # late tensors: v columns of w, vres, wproj
            nc.scalar.dma_start(
                out=vres_sb,
                in_=vres[:, :].rearrange("p (t h d) -> p t h d", t=NT, h=HPC))
            for kc in range(KC):
                nc.gpsimd.dma_start(out=wv_sb[:, kc, :],
                                    in_=w[kc * 128:(kc + 1) * 128, 1024:1536])
            for cc in range(4):
                (nc.scalar if cc % 2 == 0 else nc.gpsimd).dma_start(
                    out=wproj_sb[:, cc, :], in_=wproj[cc * 128:(cc + 1) * 128, :])
            # prewarm the GpSimd custom-op library (first partition_broadcast
            # otherwise pays a ~7.7us ucode load right when the first softmax
            # finale needs it). The src reads a wproj_sb cell so the scheduler
            # cannot hoist it ahead of the input DMA triggers on the gpsimd
            # queue — the ucode load blocks that queue for its duration.
            gwarm = constp.tile([DH, 1], BF16)
            nc.gpsimd.partition_broadcast(gwarm[:, :], wproj_sb[0:1, 3, 0:1],
                                          channels=DH)

            # ones column of v (denominator row of the av matmul)
            nc.vector.memset(v_sb[:, :, :, DH:DH + 1], 1.0)

            # ------------- phase A: fused qkv+dt projection, LN, rope ------
            # q runs kc-outer (tracks DMA chunk arrival); k and v run t-outer
            # with inline postprocessing so PSUM tiles free progressively.
            # Each projection tile is first copied to bf16 SBUF by ScalarE —
            # that copy is the tile's only PSUM reader, so the bank frees
            # ~1us after the matmuls finish, and the LN/rope math runs on
            # fast packed-bf16 SBUF DVE ops. The q/k weights are host-
            # centered, so LN needs no mean subtraction here.
            with (
                tc.tile_pool(name="qkp", bufs=1) as qkp,
                tc.tile_pool(name="psA", bufs=8, space="PSUM") as psA,
                tc.tile_pool(name="scrA", bufs=3) as scrA,
                tc.tile_pool(name="stat", bufs=4) as stat,
            ):
                qr_sb = qkp.tile([128, NT, HPC, DH], BF16)
                kr_sb = qkp.tile([128, NT, HPC, DH], BF16)

                # HAM warmup: keep the PE busy during the DMA-wait head so
                # the clock gate is at 8/8 when the real matmuls arrive.
                # HAM warmup so the real matmuls (first DMA-gated, ~9-10us
                # in) start at the full 2.4 GHz clock. Fed by a memset tile
                # (no make_identity dependency) so it starts ~6us in.
                warm = psA.tile([128, 512], F32, name="warm", tag="pp")
                for _ in range(24):
                    nc.tensor.matmul(warm[:, 0:128], wsrc[:], wsrc[:],
                                     start=True, stop=True)

                def post_qk1(ps, t, ob, on_dve=False):
                    """PSUM-freeing copy only — emitted inline with the
                    projection matmuls so banks release at copy pace."""
                    ps3 = ps.rearrange("p (h d) -> p h d", h=HPC)
                    if bias_sb is not None:
                        nc.vector.tensor_add(
                            ps[:], ps[:], bias_sb[:, ob * 512:(ob + 1) * 512])
                    xb = scrA.tile([128, HPC, DH], BF16, tag="xb", bufs=17)
                    if on_dve:
                        nc.vector.tensor_copy(xb[:], ps3)
                    else:
                        nc.scalar.activation(out=xb[:], in_=ps3, func=AF.Copy)
                    return xb

                def post_qk2(xb, t, ob):
                    """LN rstd + rope chain (DVE), deferred past the copies."""
                    sqb = scrA.tile([128, HPC, DH], BF16, tag="sqb")
                    nc.vector.tensor_mul(sqb[:], xb[:], xb[:])
                    red_q = stat.tile([128, HPC], F32, tag="red_q")
                    nc.vector.reduce_sum(out=red_q[:], in_=sqb[:], axis=AX)
                    # rstd = 1/sqrt(sum(q^2)/DH + eps); Sqrt keeps one ACT
                    # table set in phase A, DVE reciprocal leaves Exp's
                    # table untouched for the attention phase.
                    rstd = stat.tile([128, HPC], F32, tag="rstd")
                    nc.scalar.activation(out=rstd[:], in_=red_q[:], func=AF.Sqrt,
                                         scale=1.0 / DH, bias=eps_t[:])
                    nc.vector.reciprocal(rstd[:], rstd[:])
                    xr = scrA.tile([128, HPC, DH], BF16, tag="xr")
                    nc.vector.tensor_tensor(
                        out=xr[:], in0=xb[:], in1=_bcast_free(rstd[:], DH, 2)[:],
                        op=ALU.mult)
                    if ln_sb is not None:
                        gi, bi = (0, 1) if ob == 0 else (2, 3)
                        nc.vector.tensor_tensor(
                            out=xr[:], in0=xr[:],
                            in1=_bcast_free(ln_sb[:, gi, :], HPC, 1)[:],
                            op=ALU.mult)
                        nc.vector.tensor_tensor(
                            out=xr[:], in0=xr[:],
                            in1=_bcast_free(ln_sb[:, bi, :], HPC, 1)[:],
                            op=ALU.add)
                    # rope: dst = xr*cos + rot_half(xr)*sin'
                    dst = (qr_sb if ob == 0 else kr_sb)
                    rpc = _bcast_free(rpc_sb[:, t], HPC, 1)
                    rps_lo = _bcast_free(rps_sb[:, t, 0:HD], HPC, 1)
                    rps_hi = _bcast_free(rps_sb[:, t, HD:DH], HPC, 1)
                    tc_ = scrA.tile([128, HPC, DH], BF16, tag="tc")
                    nc.vector.tensor_tensor(out=tc_[:], in0=xr[:],
                                            in1=rpc[:], op=ALU.mult)
                    tm = scrA.tile([128, HPC, DH], BF16, tag="tm")
                    nc.vector.tensor_tensor(out=tm[:, :, 0:HD],
                                            in0=xr[:, :, HD:DH],
                                            in1=rps_lo[:], op=ALU.mult)
                    nc.vector.tensor_tensor(out=tm[:, :, HD:DH],
                                            in0=xr[:, :, 0:HD],
                                            in1=rps_hi[:], op=ALU.mult)
                    nc.vector.tensor_tensor(out=dst[:, t], in0=tc_[:],
                                            in1=tm[:], op=ALU.add)

                def post_v(ps, t, on_dve):
                    ps3 = ps.rearrange("p (h d) -> p h d", h=HPC)
                    if bias_sb is not None:
                        nc.vector.tensor_add(
                            ps[:], ps[:], bias_sb[:, 1024:1536])
                    if on_dve:
                        # attention-phase tiles: keep ScalarE free for exps
                        nc.vector.tensor_scalar_mul(
                            v_sb[:, t, :, 0:DH], in0=ps3, scalar1=float(lamb1))
                    else:
                        nc.scalar.activation(out=v_sb[:, t, :, 0:DH], in_=ps3,
                                             func=AF.Copy, scale=float(lamb1))
                    nc.vector.tensor_tensor(out=v_sb[:, t, :, 0:DH],
                                            in0=v_sb[:, t, :, 0:DH],
                                            in1=vres_sb[:, t], op=ALU.add)

                def proj_mms(ps, ob, t, kcs):
                    for kc in kcs:
                        wsl = (wv_sb[:, kc, :] if ob == 2
                               else w_sb[:, kc, ob * 512:(ob + 1) * 512])
                        nc.tensor.matmul(
                            ps[:],
                            xdT_sb[:, kc, t * 128:(t + 1) * 128],
                            wsl,
                            start=(kc == 0), stop=(kc == KC - 1))

                def proj_tile(ob, t, pool, tag):
                    ps = pool.tile([128, 512], F32, name=f"pt{ob}_{t}",
                                   tag=tag)
                    proj_mms(ps, ob, t, range(KC))
                    return ps

                def transpose_ob(src, dstT):
                    for j in range(HPC // 2):
                        tp = psA.tile([128, 512], F32, tag="pp")
                        tpb = tp.bitcast(BF16)
                        for t in range(NT):
                            nc.tensor.transpose(
                                tpb[:, t * 128:(t + 1) * 128],
                                src[:, t, 2 * j:2 * j + 2, :]
                                   .rearrange("p h d -> p (h d)"),
                                ident[:])
                        if dstT is qT_sb:
                            nc.scalar.activation(out=dstT[:, j, :], in_=tpb[:],
                                                 func=AF.Copy)
                        else:
                            nc.vector.tensor_copy(dstT[:, j, :], tpb[:])

                # q: bulk kc-outer (tracks DMA chunk arrival), then a
                # t-outer tail with inline posts so the DVE postprocess
                # stream is spread instead of bursting 8 chains at once.
                KS = KC
                ps_tiles = [psA.tile([128, 512], F32, name=f"pp{_t}",
                                     tag="pp")
                            for _t in range(NT)]
                for kc in range(KS):
                    for t in range(NT):
                        nc.tensor.matmul(
                            ps_tiles[t][:],
                            xdT_sb[:, kc, t * 128:(t + 1) * 128],
                            w_sb[:, kc, 0:512],
                            start=(kc == 0), stop=False)
                q_xb = []
                for t in range(NT):
                    for kc in range(KS, KC):
                        nc.tensor.matmul(
                            ps_tiles[t][:],
                            xdT_sb[:, kc, t * 128:(t + 1) * 128],
                            w_sb[:, kc, 0:512],
                            start=False, stop=(kc == KC - 1))
                    # DVE is idle here (part2 chains deferred), so alternate
                    # the copies across engines for 2x bank-release pace
                    q_xb.append(post_qk1(ps_tiles[t], t, 0,
                                         on_dve=(t % 2 == 1)))
                for t in range(NT):
                    post_qk2(q_xb[t], t, 0)
                # k (copies on ScalarE — DVE is draining the q chains),
                # then both transposes, then the first half of v
                k_xb = []
                for t in range(NT):
                    k_xb.append(post_qk1(proj_tile(1, t, psA, "pp"), t, 1))
                for t in range(NT):
                    post_qk2(k_xb[t], t, 1)
                # v tiles before the transposes: independent PE work covers
                # the DVE draining the k postprocess chains, so the
                # transposes (which need every k tile's rope done) run
                # gap-free right before attention consumes them.
                transpose_ob(qr_sb, qT_sb)
                for t in range(6):
                    post_v(proj_tile(2, t, psA, "pp"), t, on_dve=False)
                transpose_ob(kr_sb, kT_sb)
                # preload the Exp ACT table (no more Sqrts follow) so the
                # ~2.7us table switch overlaps the v tiles, not the first
                # attention exp.
                nc.scalar.activation(out=eps_t[:], in_=eps_t[:], func=AF.Exp)
                if debug:
                    rr2 = "(t p) (h d) -> p t h d"
                    nc.sync.dma_start(
                        out=dbg["d_qr"][:, :].rearrange(rr2, p=128, h=HPC),
                        in_=qr_sb)
                    nc.sync.dma_start(
                        out=dbg["d_kr"][:, :].rearrange(rr2, p=128, h=HPC),
                        in_=kr_sb)
                    nc.sync.dma_start(out=dbg["d_qT"][:, :], in_=qT_sb[:, 0, :])
                    nc.sync.dma_start(out=dbg["d_kT"][:, :], in_=kT_sb[:, 0, :])

            # ------------- attention + interleaved fillers ------------------
            # (j, qh) pair-major iteration covering both 64-dim head-halves.
            # The two halves' score matmuls are interleaved at tile_position
            # rows 0/64 so the PE row-tiles them concurrently. PE filler
            # between attention chunks: first the deferred second half of
            # the v projection (tiles 4-7, emitted in 8-matmul halves), then
            # out-projection tiles once a query half's finales land.
            with (
                tc.tile_pool(name="psS", bufs=2, space="PSUM") as psS,
                tc.tile_pool(name="psV", bufs=3, space="PSUM") as psV,
                tc.tile_pool(name="psP", bufs=1, space="PSUM") as psP,
                tc.tile_pool(name="expp", bufs=8) as expp,
                tc.tile_pool(name="nrm", bufs=2) as nrm,
                tc.tile_pool(name="outp", bufs=2) as outp,
            ):
                pairs = [(j, qh) for qh in range(2) for j in range(HPC // 2)]
                NSTEP = NT // 2  # kk steps per pair (2 key chunks per half)
                av_t = {}
                scale = 1.0 / float(np.sqrt(DH))

                def emit_sc_pair(pi, kc):
                    # one chunk-pair per step: a single [128,2,512] psum tile
                    # holds BOTH head-halves' score chunks, so their WAR
                    # clears atomically — the scheduler keeps the A/B
                    # interleave and the PE row-tiles the two 64-contraction
                    # matmuls concurrently. bufs=2 gives one step of slack
                    # between the sc matmuls and the exp of the prior step.
                    j, qh = pairs[pi]
                    sc = psS.tile([128, 2, 512], F32, tag="sc", bufs=2)
                    qs = slice(qh * 512, (qh + 1) * 512)
                    ks = slice(kc * 128, (kc + 1) * 128)
                    nc.tensor.matmul(
                        sc[:, 0, :], kT_sb[0:DH, j, ks], qT_sb[0:DH, j, qs],
                        start=True, stop=True, tile_position=(0, 0))
                    nc.tensor.matmul(
                        sc[:, 1, :], kT_sb[DH:128, j, ks],
                        qT_sb[DH:128, j, qs],
                        start=True, stop=True, tile_position=(DH, 0))
                    ex = expp.tile([128, 2, 512], BF16, tag="ex", bufs=8)
                    if pi == len(pairs) - 1:
                        # final pair: per-half exps so the drain chain
                        # (exp -> av -> finale -> out-proj) starts sooner
                        for i in range(2):
                            nc.scalar.activation(out=ex[:, i, :],
                                                 in_=sc[:, i, :],
                                                 func=AF.Exp, scale=scale)
                    else:
                        nc.scalar.activation(out=ex[:], in_=sc[:],
                                             func=AF.Exp, scale=scale)
                    if debug and pi == 0 and kc == 0:
                        nc.sync.dma_start(
                            out=dbg["d_ex"][:, 0:1024].rearrange(
                                "p (a b) -> p a b", a=2),
                            in_=ex[:])
                    return ex

                def emit_av_pair(pi, kc, ex):
                    j, qh = pairs[pi]
                    if kc == 0:
                        av_t[(pi, 0)] = psV.tile([DH + 1, 512], F32,
                                                 name=f"av{pi}_0", tag="av")
                        av_t[(pi, 1)] = psV.tile([DH + 1, 512], F32,
                                                 name=f"av{pi}_1", tag="av")
                    nc.tensor.matmul(
                        av_t[(pi, 0)][:], v_sb[:, kc, 2 * j, :],
                        ex[:, 0, :], start=(kc == 0), stop=(kc == NT - 1))
                    nc.tensor.matmul(
                        av_t[(pi, 1)][:], v_sb[:, kc, 2 * j + 1, :],
                        ex[:, 1, :], start=(kc == 0), stop=(kc == NT - 1))

                def emit_finale(pi, hh):
                    j, qh = pairs[pi]
                    ro = 64 * hh
                    av = av_t.pop((pi, hh))
                    rep = nrm.tile([DH, 2, 512], F32, tag="rep")
                    # sums row (psum partition 64) -> partition 0 SBUF on
                    # DVE (ScalarE is saturated with exps), then gpsimd-
                    # broadcast to 64 partitions, then approx-recip there
                    # (the custom DVE op misbehaves at base >= 64).
                    sums = nrm.tile([1, 512], F32, tag="sums")
                    if pi == len(pairs) - 1:
                        # ScalarE is free once the last exps retire; taking
                        # the sums copy there shortens the serialized DVE
                        # chain that gates the final out-proj units
                        nc.scalar.activation(out=sums[:], in_=av[DH:DH + 1, :],
                                             func=AF.Copy)
                    else:
                        nc.vector.tensor_copy(sums[:], av[DH:DH + 1, :])
                    # raw-av copy frees the psum bank ~1.4us after the last
                    # av matmul (instead of after the whole 3us normalize
                    # chain) so the next pair's av allocation never stalls.
                    avr = nrm.tile([DH, 512], F32, tag="avr", bufs=3)
                    nc.vector.tensor_copy(avr[:], av[0:DH, :])
                    nc.gpsimd.partition_broadcast(
                        rep[:, 0, :], sums[:], channels=DH)
                    nc.vector.reciprocal_approx_fast(
                        out=rep[:, 1, :], in_=rep[:, 0, :])
                    nc.vector.tensor_tensor(
                        out=outT_sb[ro:ro + DH, j, qh * 512:(qh + 1) * 512],
                        in0=avr[:], in1=rep[:, 1, :], op=ALU.mult)
                    if debug and pi == 0 and hh == 0:
                        avc = nrm.tile([DH + 1, 512], F32, tag="avc")
                        nc.vector.tensor_copy(avc[:], av[:])
                        nc.sync.dma_start(out=dbg["d_av"][:, :], in_=avc)
                        nc.sync.dma_start(out=dbg["d_rcp"][:, :],
                                          in_=rep[0:1, 1, :])
                        nc.sync.dma_start(out=dbg["d_rep"][:, :],
                                          in_=rep[:, 1, :])

                # qh0 units (t 0..3) run fully as attention fillers. qh1
                # units (t 4..7) run in two stages: cc 0..2 partial-summed
                # to SBUF f32 during the last pair (j0..2 finales are in),
                # then one cc=3 matmul + DVE add after the last finale.
                proj_units = [(t, oh) for t in range(NT) for oh in range(2)]
                stg_t = {}
                prt = {}
                state = {"emitted": 0, "finales": 0, "vdef": 0, "partial": 8}

                def unit_mms(pp, t, oh, ccs, start_cc, stop_cc):
                    for cc in ccs:
                        nc.tensor.matmul(
                            pp[:],
                            outT_sb[:, cc, t * 128:(t + 1) * 128],
                            wproj_sb[:, cc, oh * 512:(oh + 1) * 512],
                            start=(cc == start_cc), stop=(cc == stop_cc))

                def emit_proj_unit():
                    t, oh = proj_units[state["emitted"]]
                    state["emitted"] += 1
                    if oh == 0:
                        stg_t[t] = outp.tile([128, C], BF16, name=f"stg{t}",
                                             tag="stg")
                    pp = psP.tile([128, 512], F32, tag="pp2")
                    unit_mms(pp, t, oh, range(4), 0, 3)
                    nc.vector.tensor_copy(
                        stg_t[t][:, oh * 512:(oh + 1) * 512], pp[:])
                    if oh == 1:
                        nc.sync.dma_start(out=out[t * 128:(t + 1) * 128, :],
                                          in_=stg_t.pop(t))

                def emit_partial_unit():
                    # t=4,5 units: cc0..2 partial -> SBUF f32 (final = DVE add)
                    t, oh = proj_units[state["partial"]]
                    state["partial"] += 1
                    pp = psP.tile([128, 512], F32, tag="pp2")
                    unit_mms(pp, t, oh, range(3), 0, 2)
                    prt[(t, oh)] = outp.tile([128, 512], F32,
                                             name=f"prt{t}_{oh}", tag="prt",
                                             bufs=4)
                    nc.vector.tensor_copy(prt[(t, oh)][:], pp[:])

                prt_ps = {}

                def emit_psum_partials():
                    # t=6,7 units: cc0..2 stay resident in psS tiles (their
                    # sc traffic is over); final = one accumulating matmul +
                    # a ScalarE copy, so the tail splits across DVE + ScalarE
                    for t in (6, 7):
                        pt = psS.tile([128, 2, 512], F32, name=f"prtps{t}",
                                      tag="sc", bufs=2)
                        for oh in range(2):
                            unit_mms(pt[:, oh, :], t, oh, range(3), 0, -1)
                        prt_ps[t] = pt

                def emit_final_unit(t, oh):
                    if oh == 0:
                        stg_t[t] = outp.tile([128, C], BF16, name=f"stg{t}",
                                             tag="stg")
                    if t in prt_ps:
                        pt = prt_ps[t]
                        unit_mms(pt[:, oh, :], t, oh, [3], -1, 3)
                        nc.scalar.activation(
                            out=stg_t[t][:, oh * 512:(oh + 1) * 512],
                            in_=pt[:, oh, :], func=AF.Copy)
                    else:
                        # the av banks are all retired by now — rotating the
                        # final-unit psums through psV avoids serializing on
                        # the single psP bank
                        pp = psV.tile([128, 512], F32, tag="av",
                                      name=f"fu{t}_{oh}")
                        unit_mms(pp, t, oh, [3], 3, 3)
                        nc.vector.tensor_tensor(
                            out=stg_t[t][:, oh * 512:(oh + 1) * 512],
                            in0=pp[:], in1=prt.pop((t, oh))[:], op=ALU.add)
                    if oh == 1:
                        nc.sync.dma_start(out=out[t * 128:(t + 1) * 128, :],
                                          in_=stg_t.pop(t))

                # deferred v tiles 4-7 spread over attention steps 0-5;
                # deferred v tiles 4-7 spread as half-tile fillers over the
                # first chunk-steps; tile T's second half lands at step
                # 2(T-4)+1, well before its first av reader (kc=T) is
                # emitted at step T+6 with the lookahead of 6.
                vplan = [(6, 0), (6, 1), (7, 0), (7, 1)]
                vps = {}

                def emit_filler():
                    if state["vdef"] < len(vplan):
                        t, half = vplan[state["vdef"]]
                        state["vdef"] += 1
                        if half == 0:
                            vps[t] = psP.tile([128, 512], F32,
                                              name=f"vt{t}", tag="pp2")
                            proj_mms(vps[t], 2, t, range(0, KC // 2))
                        else:
                            proj_mms(vps[t], 2, t, range(KC // 2, KC))
                            post_v(vps.pop(t), t, on_dve=True)
                        return
                    if state["finales"] >= 8 and state["emitted"] < 8:
                        emit_proj_unit()
                        return
                    if state["finales"] >= 14 and state["partial"] < 12:
                        emit_partial_unit()
                        if state["partial"] < 12:
                            emit_partial_unit()

                # pipeline: sc/exp run 6 chunk-steps ahead of av; finales
                # (all DVE/GpSimd) are emitted as soon as the last av lands.
                steps = [(pi, kc) for pi in range(len(pairs))
                         for kc in range(NT)]
                exq = []      # (pi, kc, ex) awaiting av emission
                fill_tick = 0
                for (pi, kc) in steps:
                    # last pair: drop the av lookahead to 1 so the drain
                    # chain (av -> finale -> out-proj tail) starts sooner
                    look = 1 if pi == len(pairs) - 1 else 6
                    while len(exq) >= look:
                        api, akc, aex = exq.pop(0)
                        emit_av_pair(api, akc, aex)
                        if akc == NT - 1:
                            emit_finale(api, 0)
                            emit_finale(api, 1)
                            state["finales"] += 2
                    exq.append((pi, kc, emit_sc_pair(pi, kc)))
                    # fillers are ~1.7-1us of PE work; one per two chunk-
                    # steps keeps the PE just above the exp pace
                    fill_tick += 1
                    if state["vdef"] < len(vplan) or fill_tick % 2 == 0:
                        emit_filler()
                for (api, akc, aex) in exq:
                    emit_av_pair(api, akc, aex)
                    if akc == NT - 1:
                        emit_finale(api, 0)
                        emit_finale(api, 1)
                        state["finales"] += 2
                if debug:
                    nc.sync.dma_start(out=dbg["d_outT"][:, :],
                                      in_=outT_sb[:, 0, :])
                    nc.sync.dma_start(
                        out=dbg["d_v"][:, :].rearrange(
                            "(t p) (h d) -> p t h d", p=128, h=HPC),
                        in_=v_sb)
                while state["emitted"] < 8:
                    emit_proj_unit()
                while state["partial"] < 12:
                    emit_partial_unit()
                emit_psum_partials()
                # interleave DVE-add units (t4,5) with ScalarE-copy units
                # (t6,7) so the tail splits across both engines
                for t in (4, 6, 5, 7):
                    emit_final_unit(t, 0)
                    emit_final_unit(t, 1)

    nc.finalize()
    return nc


_CACHE = {}
_LAST_RES = None


def _bf16(a):
    return np.ascontiguousarray(a.astype(ml_dtypes.bfloat16))


def kernel(x, rope, delta_t_emb, v_residual_v1, Wqkv, bqkv, Wdt, bdt,
           qn_g, qn_b, kn_g, kn_b, lamb1, lamb2, Wproj, bproj):
    x = np.asarray(x, np.float32)
    rope = np.ascontiguousarray(np.asarray(rope, np.float32))
    delta_t_emb = np.asarray(delta_t_emb, np.float32)
    v_residual_v1 = np.asarray(v_residual_v1, np.float32)
    Wqkv = np.asarray(Wqkv, np.float32)
    Wdt = np.asarray(Wdt, np.float32)
    Wproj = np.asarray(Wproj, np.float32)
    bias = np.asarray(bqkv, np.float32) + np.asarray(bdt, np.float32)
    l1 = float(np.asarray(lamb1)); l2 = float(np.asarray(lamb2))
    qn_g = np.asarray(qn_g, np.float32); qn_b = np.asarray(qn_b, np.float32)
    kn_g = np.asarray(kn_g, np.float32); kn_b = np.asarray(kn_b, np.float32)

    has_bias = bool(np.any(bias))
    has_ln = not (np.all(qn_g == 1.0) and np.all(qn_b == 0.0)
                  and np.all(kn_g == 1.0) and np.all(kn_b == 0.0))

    dbgf = bool(int(os.environ.get("KERNEL_DEBUG", "0")))
    key = (l1, has_bias, has_ln, dbgf)
    if key not in _CACHE:
        _CACHE[key] = build(l1, has_bias, has_ln, debug=dbgf)
    nc = _CACHE[key]

    # host-prepared rope tables in SBUF layout [p, t*DH]:
    # cos table and sign-folded sin table (rotate_half absorbed:
    # out = x*cos + rot(x)*sin' with sin' = [-sin_lo || sin_hi]).
    sin = rope[:, 0:DH]; cos = rope[:, DH:2 * DH]
    sinp = np.concatenate([-sin[:, 0:HD], sin[:, HD:DH]], axis=1)

    def _ptile(a):  # [N, DH] -> [128, NT*DH] with n = t*128 + p
        return _bf16(a.reshape(NT, 128, DH).transpose(1, 0, 2).reshape(128, -1))

    cos_p = _ptile(cos)
    sin_p = _ptile(sinp)

    in_maps = []
    for c in range(8):
        b = c // 2
        g = c % 2
        rsl = slice(g * 512, (g + 1) * 512)
        w_core = np.concatenate([
            np.concatenate([Wqkv[rsl], Wqkv[C:][rsl], Wqkv[2 * C:][rsl]], 0).T,
            np.concatenate([Wdt[rsl], Wdt[C:][rsl], Wdt[2 * C:][rsl]], 0).T,
        ], axis=0)
        w_core = np.ascontiguousarray(w_core)
        bc = np.concatenate([bias[rsl], bias[C:][rsl], bias[2 * C:][rsl]])
        bc = bc.astype(np.float32).copy()
        # fold LN mean-centering into the q/k weight+bias head blocks
        # (exact: (x@W + b)@C = x@(W@C) + b@C with C = I - J/64)
        for ob in range(2):
            for h in range(HPC):
                sl = slice(ob * 512 + h * DH, ob * 512 + (h + 1) * DH)
                w_core[:, sl] -= w_core[:, sl].mean(axis=1, keepdims=True)
                bc[sl] -= bc[sl].mean()
        # vres in SBUF layout [p, t, h, d] flattened
        vr = (l2 * v_residual_v1[b, g * 8:(g + 1) * 8]).transpose(1, 0, 2)
        vr = vr.reshape(NT, 128, HPC, DH).transpose(1, 0, 2, 3).reshape(128, -1)
        # rotate each core's contraction-chunk order (sum order is free):
        # the 8 SPMD cores otherwise request the SAME chunk simultaneously
        # and serialize on HBM during the critical early stream
        rot = (2 * c) % KC
        perm = [(k + rot) % KC for k in range(KC)]
        xdT_full = np.concatenate([x[b].T, delta_t_emb[b].T], 0)
        xdT_rot = np.concatenate([xdT_full[128 * k:128 * (k + 1)]
                                  for k in perm], 0)
        w_rot = np.concatenate([w_core[128 * k:128 * (k + 1)]
                                for k in perm], 0)
        m = {
            "xdT": _bf16(xdT_rot),
            "w": _bf16(w_rot),
            "vres": _bf16(vr),
            "ropec": cos_p,
            "ropes": sin_p,
            "wproj": _bf16(Wproj[:, rsl].T),
        }
        if has_bias:
            m["biasd"] = np.ascontiguousarray(bc[None, :].astype(np.float32))
        if has_ln:
            m["lnp"] = _bf16(np.stack([qn_g, qn_b, kn_g, kn_b], 0))
        in_maps.append(m)

    trace = bool(int(os.environ.get("KERNEL_TRACE", "0")))
    res = run_bass_kernel_spmd(nc, in_maps, core_ids=list(range(8)), trace=trace)
    global _LAST_RES
    _LAST_RES = res
    if trace and res.exec_time_ns is not None:
        print(f"HW exec time: {res.exec_time_ns} ns")
        kernel.last_exec_time_ns = res.exec_time_ns
        kernel.last_results = res

    out = np.empty((B, N, C), np.float32)
    for b in range(B):
        out[b] = (res.results[2 * b]["out"].astype(np.float32)
                  + res.results[2 * b + 1]["out"].astype(np.float32))
    bproj = np.asarray(bproj, np.float32)
    if np.any(bproj):
        out += bproj[None, None, :]
    return out


# revision 72
# speedup vs baseline: 1.1643x; 1.0019x over previous
"""Trainium2 Bass kernel for nn_AttentionBlock (B=4, N=1024, C=1024, H=16).

Sharding: 8 cores = 4 batches x 2 head-groups (8 heads each). Each core
computes its batch's tokens for its 8 heads end-to-end (fused qkv+delta
projection, qk-LayerNorm, RoPE, softmax attention with value-residual mix,
and a partial output projection over its head columns). The host sums the
two partial projections per batch.

Measured ~202us on 8 trn2 cores (baseline lineage: 452us f32r ->
253us bf16 -> 202us this version). Key optimizations:
- all matmul operands bf16; LN mean-centering folded into the q/k weight
  columns on the host (exact: (x@W)@C = x@(W@C) with C = I - J/64), so the
  in-kernel LN is just rstd scaling (fp8 DoubleRow was tried and rejected:
  attention-weight/v quantization error does NOT average down — it scales
  with the same sqrt(sum a^2) factor as the signal -> ~5e-2 rel err);
- rope/vres tables are host-permuted to the exact SBUF layout so their
  DMAs are contiguous (128 descriptors, not 8192) and ride the idle
  scalar queue; w streams q+k columns before v columns (separate wv tile
  to avoid false chunk-row deps), early chunks alternate queues; each
  core's contraction chunks are host-rotated by 2*core_id so the 8 SPMD
  cores don't request the same HBM region simultaneously;
- score matmuls for the two 64-dim head-halves live in ONE [128,2,512]
  psum tile (their WAR clears atomically, so the scheduler keeps the A/B
  interleave) and run concurrently via tile_position row tiling;
- chunk-grained attention pipeline: per step one sc pair + one 1024-wide
  exp, av matmuls trail by 6 steps, psS bufs=2 gives the exp stream one
  step of slack; psV bufs=3 plus an early raw-av copy (normalize multiply
  reads the copy) so av psum banks free ~1.4us after their last matmul;
- warmup matmuls during the DMA-wait head keep the PE HAM clock at 8/8;
  the GpSimd broadcast ucode library is prewarmed behind the input DMA
  triggers (first use otherwise costs ~7.7us on the gpsimd queue);
- softmax normalize: ones-column denominator row -> GpSimd broadcast ->
  DVE approx-reciprocal -> one multiply; the Exp ACT table is preloaded
  during phase A so no table switch gates the first attention exp;
- tail: qh1 output-projection units are partial-summed (cc 0..2) during
  the last pair — two via SBUF f32 partials (DVE add) and two resident in
  retired score psum banks (ScalarE copy) — so only one matmul plus one
  copy/add per unit remains after the final softmax finale.
"""
import os
import sys

sys.path.insert(0, "/opt/trn_rl_repo")

import numpy as np
import ml_dtypes

import concourse.bass as bass
import concourse.bacc as bacc
import concourse.tile as tile
from concourse import mybir
from concourse.bass_utils import run_bass_kernel_spmd
from concourse.masks import make_identity

F32 = mybir.dt.float32
BF16 = mybir.dt.bfloat16

B, N, C, H = 4, 1024, 1024, 16
DH = C // H            # 64
HD = DH // 2           # 32
HPC = 8                # heads per core
NT = N // 128          # 8 token tiles
KC = (2 * C) // 128    # 16 contraction chunks for fused qkv+dt
EPS = 1e-5
AX = mybir.AxisListType.X
ALU = mybir.AluOpType
AF = mybir.ActivationFunctionType


def _bcast_free(ap, n, axis_pos=1):
    """Insert a step-0 free dim of size n at axis_pos of an AP."""
    new = list(ap.ap)
    new.insert(axis_pos, [0, n])
    return bass.AP(tensor=ap.tensor, offset=ap.offset, ap=new)


def _bcast_part(ap, n):
    """Partition-broadcast AP (step-0 partition dim) for DMA use."""
    return bass.AP(tensor=ap.tensor, offset=ap.offset, ap=[[0, n]] + list(ap.ap[1:]))


def build(lamb1, has_bias, has_ln, debug=False):
    """Build the single-core SPMD program.

    lamb1: python float (v-residual own-value weight; the residual weight
    lamb2 is folded into the host-prescaled vres input).
    has_bias: combined qkv+dt bias is nonzero -> biasd input present.
    has_ln: any qk-LayerNorm affine param nontrivial -> lnp input present.
    """
    nc = bacc.Bacc("TRN2", target_bir_lowering=False)

    xdT = nc.dram_tensor("xdT", [2 * C, N], BF16, kind="ExternalInput")
    w = nc.dram_tensor("w", [2 * C, 3 * HPC * DH], BF16, kind="ExternalInput")
    # vres / rope tables come host-permuted into SBUF layout (p-major).
    vres = nc.dram_tensor("vres", [128, NT * HPC * DH], BF16, kind="ExternalInput")
    ropec = nc.dram_tensor("ropec", [128, NT * DH], BF16, kind="ExternalInput")
    ropes = nc.dram_tensor("ropes", [128, NT * DH], BF16, kind="ExternalInput")
    wproj = nc.dram_tensor("wproj", [HPC * DH, C], BF16, kind="ExternalInput")
    biasd = None
    if has_bias:
        biasd = nc.dram_tensor("biasd", [1, 3 * HPC * DH], F32, kind="ExternalInput")
    lnpd = None
    if has_ln:
        lnpd = nc.dram_tensor("lnp", [4, DH], BF16, kind="ExternalInput")
    out = nc.dram_tensor("out", [N, C], BF16, kind="ExternalOutput")
    dbg = {}
    if debug:
        for nm, shp, dt in [
                ("d_qr", [N, HPC * DH], BF16), ("d_kr", [N, HPC * DH], BF16),
                ("d_kT", [128, N], BF16), ("d_qT", [128, N], BF16),
                ("d_v", [N, HPC * (DH + 1)], BF16),
                ("d_ex", [128, 1024], BF16), ("d_av", [DH + 1, 512], F32),
                ("d_rcp", [1, 512], F32), ("d_rep", [DH, 512], F32),
                ("d_outT", [128, N], BF16)]:
            dbg[nm] = nc.dram_tensor(nm, shp, dt, kind="ExternalOutput")

    with tile.TileContext(nc) as tc:
        with (
            tc.tile_pool(name="const", bufs=1) as constp,
            tc.tile_pool(name="longp", bufs=1) as longp,
        ):
            ident = constp.tile([128, 128], BF16)
            make_identity(nc, ident)
            eps_t = constp.tile([128, 1], F32)
            nc.vector.memset(eps_t, EPS)
            wsrc = constp.tile([128, 128], BF16)
            nc.vector.memset(wsrc, 0.5)
            ones_r = constp.tile([1, DH], F32)
            nc.vector.memset(ones_r, 1.0)

            bias_sb = None
            if biasd is not None:
                bias_sb = constp.tile([128, 3 * HPC * DH], F32)
                nc.scalar.dma_start(out=bias_sb, in_=_bcast_part(biasd[:, :], 128))
            ln_sb = None
            if lnpd is not None:
                ln_sb = constp.tile([128, 4, DH], BF16)
                nc.scalar.dma_start(out=ln_sb, in_=_bcast_part(lnpd[:, :], 128))

            # big persistent SBUF tensors
            xdT_sb = longp.tile([128, KC, N], BF16)
            w_sb = longp.tile([128, KC, 2 * HPC * DH], BF16)
            wv_sb = longp.tile([128, KC, HPC * DH], BF16)
            rpc_sb = longp.tile([128, NT, DH], BF16)
            rps_sb = longp.tile([128, NT, DH], BF16)
            vres_sb = longp.tile([128, NT, HPC, DH], BF16)
            v_sb = longp.tile([128, NT, HPC, DH + 1], BF16)
            qT_sb = longp.tile([128, HPC // 2, N], BF16)
            kT_sb = longp.tile([128, HPC // 2, N], BF16)
            outT_sb = longp.tile([128, HPC // 2, N], BF16)
            wproj_sb = longp.tile([128, 4, C], BF16)

            # ---- input DMAs. Critical order: the tiny rope tables and the
            # first xdT / w(q+k cols) chunks lead; v-projection columns,
            # vres and wproj stream later (first needed mid-phase-A).
            # sync queue: xdT + rope; gpsimd queue: w. ~5MB each.
            nc.sync.dma_start(out=xdT_sb[:, 0, 0:256], in_=xdT[0:128, 0:256])
            nc.gpsimd.dma_start(out=w_sb[:, 0, 0:1024], in_=w[0:128, 0:1024])
            nc.scalar.dma_start(out=rpc_sb,
                                in_=ropec[:, :].rearrange("p (t d) -> p t d", t=NT))
            nc.scalar.dma_start(out=rps_sb,
                                in_=ropes[:, :].rearrange("p (t d) -> p t d", t=NT))
            nc.sync.dma_start(out=xdT_sb[:, 0, 256:N], in_=xdT[0:128, 256:N])
            # first chunks alternate queues so the DMA-paced q-bulk start
            # gets both tensors' early chunks at ~2x cadence
            for kc in range(1, 5):
                qa, qb = ((nc.sync, nc.gpsimd) if kc % 2 == 1
                          else (nc.gpsimd, nc.sync))
                qa.dma_start(out=xdT_sb[:, kc, :],
                             in_=xdT[kc * 128:(kc + 1) * 128, :])
                qb.dma_start(out=w_sb[:, kc, 0:1024],
                             in_=w[kc * 128:(kc + 1) * 128, 0:1024])
            for kc in range(5, KC):
                nc.sync.dma_start(out=xdT_sb[:, kc, :],
                                  in_=xdT[kc * 128:(kc + 1) * 128, :])
                nc.gpsimd.dma_start(out=w_sb[:, kc, 0:1024],
                                    in_=w[kc * 128:(kc + 1) * 128, 0:1024])
            # late tensors: v columns of w, vres, wproj
            nc.scalar.dma_start(
                out=vres_sb,
                in_=vres[:, :].rearrange("p (t h d) -> p t h d", t=NT, h=HPC))
            for kc in range(KC):
                nc.gpsimd.dma_start(out=wv_sb[:, kc, :],
                                    in_=w[kc * 128:(kc + 1) * 128, 1024:1536])
            for cc in range(4):
                (nc.scalar if cc % 2 == 0 else nc.gpsimd).dma_start(
                    out=wproj_sb[:, cc, :], in_=wproj[cc * 128:(cc + 1) * 128, :])
            # prewarm the GpSimd custom-op library (first partition_broadcast
            # otherwise pays a ~7.7us ucode load right when the first softmax
            # finale needs it). The src reads a wproj_sb cell so the scheduler
            # cannot hoist it ahead of the input DMA triggers on the gpsimd
            # queue — the ucode load blocks that queue for its duration.
            gwarm = constp.tile([DH, 1], BF16)
            nc.gpsimd.partition_broadcast(gwarm[:, :], wproj_sb[0:1, 3, 0:1],
                                          channels=DH)

            # ones column of v (denominator row of the av matmul)
            nc.vector.memset(v_sb[:, :, :, DH:DH + 1], 1.0)

            # ------------- phase A: fused qkv+dt projection, LN, rope ------
            # q runs kc-outer (tracks DMA chunk arrival); k and v run t-outer
            # with inline postprocessing so PSUM tiles free progressively.
            # Each projection tile is first copied to bf16 SBUF by ScalarE —
            # that copy is the tile's only PSUM reader, so the bank frees
            # ~1us after the matmuls finish, and the LN/rope math runs on
            # fast packed-bf16 SBUF DVE ops. The q/k weights are host-
            # centered, so LN needs no mean subtraction here.
            with (
                tc.tile_pool(name="qkp", bufs=1) as qkp,
                tc.tile_pool(name="psA", bufs=8, space="PSUM") as psA,
                tc.tile_pool(name="scrA", bufs=3) as scrA,
                tc.tile_pool(name="stat", bufs=4) as stat,
            ):
                qr_sb = qkp.tile([128, NT, HPC, DH], BF16)
                kr_sb = qkp.tile([128, NT, HPC, DH], BF16)

                # HAM warmup: keep the PE busy during the DMA-wait head so
                # the clock gate is at 8/8 when the real matmuls arrive.
                # HAM warmup so the real matmuls (first DMA-gated, ~9-10us
                # in) start at the full 2.4 GHz clock. Fed by a memset tile
                # (no make_identity dependency) so it starts ~6us in.
                warm = psA.tile([128, 512], F32, name="warm", tag="pp")
                for _ in range(24):
                    nc.tensor.matmul(warm[:, 0:128], wsrc[:], wsrc[:],
                                     start=True, stop=True)

                def post_qk1(ps, t, ob, on_dve=False):
                    """PSUM-freeing copy only — emitted inline with the
                    projection matmuls so banks release at copy pace."""
                    ps3 = ps.rearrange("p (h d) -> p h d", h=HPC)
                    if bias_sb is not None:
                        nc.vector.tensor_add(
                            ps[:], ps[:], bias_sb[:, ob * 512:(ob + 1) * 512])
                    xb = scrA.tile([128, HPC, DH], BF16, tag="xb", bufs=17)
                    if on_dve:
                        nc.vector.tensor_copy(xb[:], ps3)
                    else:
                        nc.scalar.activation(out=xb[:], in_=ps3, func=AF.Copy)
                    return xb

                def post_qk2(xb, t, ob):
                    """LN rstd + rope chain (DVE), deferred past the copies."""
                    sqb = scrA.tile([128, HPC, DH], BF16, tag="sqb")
                    nc.vector.tensor_mul(sqb[:], xb[:], xb[:])
                    red_q = stat.tile([128, HPC], F32, tag="red_q")
                    nc.vector.reduce_sum(out=red_q[:], in_=sqb[:], axis=AX)
                    # rstd = 1/sqrt(sum(q^2)/DH + eps); Sqrt keeps one ACT
                    # table set in phase A, DVE reciprocal leaves Exp's
                    # table untouched for the attention phase.
                    rstd = stat.tile([128, HPC], F32, tag="rstd")
                    nc.scalar.activation(out=rstd[:], in_=red_q[:], func=AF.Sqrt,
                                         scale=1.0 / DH, bias=eps_t[:])
                    nc.vector.reciprocal(rstd[:], rstd[:])
                    xr = scrA.tile([128, HPC, DH], BF16, tag="xr")
                    nc.vector.tensor_tensor(
                        out=xr[:], in0=xb[:], in1=_bcast_free(rstd[:], DH, 2)[:],
                        op=ALU.mult)
                    if ln_sb is not None:
                        gi, bi = (0, 1) if ob == 0 else (2, 3)
                        nc.vector.tensor_tensor(
                            out=xr[:], in0=xr[:],
                            in1=_bcast_free(ln_sb[:, gi, :], HPC, 1)[:],
                            op=ALU.mult)
                        nc.vector.tensor_tensor(
                            out=xr[:], in0=xr[:],
                            in1=_bcast_free(ln_sb[:, bi, :], HPC, 1)[:],
                            op=ALU.add)
                    # rope: dst = xr*cos + rot_half(xr)*sin'
                    dst = (qr_sb if ob == 0 else kr_sb)
                    rpc = _bcast_free(rpc_sb[:, t], HPC, 1)
                    rps_lo = _bcast_free(rps_sb[:, t, 0:HD], HPC, 1)
                    rps_hi = _bcast_free(rps_sb[:, t, HD:DH], HPC, 1)
                    tc_ = scrA.tile([128, HPC, DH], BF16, tag="tc")
                    nc.vector.tensor_tensor(out=tc_[:], in0=xr[:],
                                            in1=rpc[:], op=ALU.mult)
                    tm = scrA.tile([128, HPC, DH], BF16, tag="tm")
                    nc.vector.tensor_tensor(out=tm[:, :, 0:HD],
                                            in0=xr[:, :, HD:DH],
                                            in1=rps_lo[:], op=ALU.mult)
                    nc.vector.tensor_tensor(out=tm[:, :, HD:DH],
                                            in0=xr[:, :, 0:HD],
                                            in1=rps_hi[:], op=ALU.mult)
                    nc.vector.tensor_tensor(out=dst[:, t], in0=tc_[:],
                                            in1=tm[:], op=ALU.add)

                def post_v(ps, t, on_dve):
                    ps3 = ps.rearrange("p (h d) -> p h d", h=HPC)
                    if bias_sb is not None:
                        nc.vector.tensor_add(
                            ps[:], ps[:], bias_sb[:, 1024:1536])
                    if on_dve:
                        # attention-phase tiles: keep ScalarE free for exps
                        nc.vector.tensor_scalar_mul(
                            v_sb[:, t, :, 0:DH], in0=ps3, scalar1=float(lamb1))
                    else:
                        nc.scalar.activation(out=v_sb[:, t, :, 0:DH], in_=ps3,
                                             func=AF.Copy, scale=float(lamb1))
                    nc.vector.tensor_tensor(out=v_sb[:, t, :, 0:DH],
                                            in0=v_sb[:, t, :, 0:DH],
                                            in1=vres_sb[:, t], op=ALU.add)

                def proj_mms(ps, ob, t, kcs):
                    for kc in kcs:
                        wsl = (wv_sb[:, kc, :] if ob == 2
                               else w_sb[:, kc, ob * 512:(ob + 1) * 512])
                        nc.tensor.matmul(
                            ps[:],
                            xdT_sb[:, kc, t * 128:(t + 1) * 128],
                            wsl,
                            start=(kc == 0), stop=(kc == KC - 1))

                def proj_tile(ob, t, pool, tag):
                    ps = pool.tile([128, 512], F32, name=f"pt{ob}_{t}",
                                   tag=tag)
                    proj_mms(ps, ob, t, range(KC))
                    return ps

                def transpose_ob(src, dstT):
                    for j in range(HPC // 2):
                        tp = psA.tile([128, 512], F32, tag="pp")
                        tpb = tp.bitcast(BF16)
                        for t in range(NT):
                            nc.tensor.transpose(
                                tpb[:, t * 128:(t + 1) * 128],
                                src[:, t, 2 * j:2 * j + 2, :]
                                   .rearrange("p h d -> p (h d)"),
                                ident[:])
                        if dstT is qT_sb:
                            nc.scalar.activation(out=dstT[:, j, :], in_=tpb[:],
                                                 func=AF.Copy)
                        else:
                            nc.vector.tensor_copy(dstT[:, j, :], tpb[:])

                # q: bulk kc-outer (tracks DMA chunk arrival), then a
                # t-outer tail with inline posts so the DVE postprocess
                # stream is spread instead of bursting 8 chains at once.
                KS = KC
                ps_tiles = [psA.tile([128, 512], F32, name=f"pp{_t}",
                                     tag="pp")
                            for _t in range(NT)]
                for kc in range(KS):
                    for t in range(NT):
                        nc.tensor.matmul(
                            ps_tiles[t][:],
                            xdT_sb[:, kc, t * 128:(t + 1) * 128],
                            w_sb[:, kc, 0:512],
                            start=(kc == 0), stop=False)
                q_xb = []
                for t in range(NT):
                    for kc in range(KS, KC):
                        nc.tensor.matmul(
                            ps_tiles[t][:],
                            xdT_sb[:, kc, t * 128:(t + 1) * 128],
                            w_sb[:, kc, 0:512],
                            start=False, stop=(kc == KC - 1))
                    # DVE is idle here (part2 chains deferred), so alternate
                    # the copies across engines for 2x bank-release pace
                    q_xb.append(post_qk1(ps_tiles[t], t, 0,
                                         on_dve=(t % 2 == 1)))
                for t in range(NT):
                    post_qk2(q_xb[t], t, 0)
                # k (copies on ScalarE — DVE is draining the q chains),
                # then both transposes, then the first half of v
                k_xb = []
                for t in range(NT):
                    k_xb.append(post_qk1(proj_tile(1, t, psA, "pp"), t, 1))
                for t in range(NT):
                    post_qk2(k_xb[t], t, 1)
                # v tiles before the transposes: independent PE work covers
                # the DVE draining the k postprocess chains, so the
                # transposes (which need every k tile's rope done) run
                # gap-free right before attention consumes them.
                transpose_ob(qr_sb, qT_sb)
                for t in range(6):
                    post_v(proj_tile(2, t, psA, "pp"), t, on_dve=False)
                transpose_ob(kr_sb, kT_sb)
                # preload the Exp ACT table (no more Sqrts follow) so the
                # ~2.7us table switch overlaps the v tiles, not the first
                # attention exp.
                nc.scalar.activation(out=eps_t[:], in_=eps_t[:], func=AF.Exp)
                if debug:
                    rr2 = "(t p) (h d) -> p t h d"
                    nc.sync.dma_start(
                        out=dbg["d_qr"][:, :].rearrange(rr2, p=128, h=HPC),
                        in_=qr_sb)
                    nc.sync.dma_start(
                        out=dbg["d_kr"][:, :].rearrange(rr2, p=128, h=HPC),
                        in_=kr_sb)
                    nc.sync.dma_start(out=dbg["d_qT"][:, :], in_=qT_sb[:, 0, :])
                    nc.sync.dma_start(out=dbg["d_kT"][:, :], in_=kT_sb[:, 0, :])

            # ------------- attention + interleaved fillers ------------------
            # (j, qh) pair-major iteration covering both 64-dim head-halves.
            # The two halves' score matmuls are interleaved at tile_position
            # rows 0/64 so the PE row-tiles them concurrently. PE filler
            # between attention chunks: first the deferred second half of
            # the v projection (tiles 4-7, emitted in 8-matmul halves), then
            # out-projection tiles once a query half's finales land.
            with (
                tc.tile_pool(name="psS", bufs=2, space="PSUM") as psS,
                tc.tile_pool(name="psV", bufs=3, space="PSUM") as psV,
                tc.tile_pool(name="psP", bufs=1, space="PSUM") as psP,
                tc.tile_pool(name="expp", bufs=8) as expp,
                tc.tile_pool(name="nrm", bufs=2) as nrm,
                tc.tile_pool(name="outp", bufs=2) as outp,
            ):
                pairs = [(j, qh) for qh in range(2) for j in range(HPC // 2)]
                NSTEP = NT // 2  # kk steps per pair (2 key chunks per half)
                av_t = {}
                scale = 1.0 / float(np.sqrt(DH))

                def emit_sc_pair(pi, kc):
                    # one chunk-pair per step: a single [128,2,512] psum tile
                    # holds BOTH head-halves' score chunks, so their WAR
                    # clears atomically — the scheduler keeps the A/B
                    # interleave and the PE row-tiles the two 64-contraction
                    # matmuls concurrently. bufs=2 gives one step of slack
                    # between the sc matmuls and the exp of the prior step.
                    j, qh = pairs[pi]
                    sc = psS.tile([128, 2, 512], F32, tag="sc", bufs=2)
                    qs = slice(qh * 512, (qh + 1) * 512)
                    ks = slice(kc * 128, (kc + 1) * 128)
                    nc.tensor.matmul(
                        sc[:, 0, :], kT_sb[0:DH, j, ks], qT_sb[0:DH, j, qs],
                        start=True, stop=True, tile_position=(0, 0))
                    nc.tensor.matmul(
                        sc[:, 1, :], kT_sb[DH:128, j, ks],
                        qT_sb[DH:128, j, qs],
                        start=True, stop=True, tile_position=(DH, 0))
                    ex = expp.tile([128, 2, 512], BF16, tag="ex", bufs=8)
                    if pi == len(pairs) - 1:
                        # final pair: per-half exps so the drain chain
                        # (exp -> av -> finale -> out-proj) starts sooner
                        for i in range(2):
                            nc.scalar.activation(out=ex[:, i, :],
                                                 in_=sc[:, i, :],
                                                 func=AF.Exp, scale=scale)
                    else:
                        nc.scalar.activation(out=ex[:], in_=sc[:],
                                             func=AF.Exp, scale=scale)
                    if debug and pi == 0 and kc == 0:
                        nc.sync.dma_start(
                            out=dbg["d_ex"][:, 0:1024].rearrange(
                                "p (a b) -> p a b", a=2),
                            in_=ex[:])
                    return ex

                def emit_av_pair(pi, kc, ex):
                    j, qh = pairs[pi]
                    if kc == 0:
                        av_t[(pi, 0)] = psV.tile([DH + 1, 512], F32,
                                                 name=f"av{pi}_0", tag="av")
                        av_t[(pi, 1)] = psV.tile([DH + 1, 512], F32,
                                                 name=f"av{pi}_1", tag="av")
                    nc.tensor.matmul(
                        av_t[(pi, 0)][:], v_sb[:, kc, 2 * j, :],
                        ex[:, 0, :], start=(kc == 0), stop=(kc == NT - 1))
                    nc.tensor.matmul(
                        av_t[(pi, 1)][:], v_sb[:, kc, 2 * j + 1, :],
                        ex[:, 1, :], start=(kc == 0), stop=(kc == NT - 1))

                def emit_finale(pi, hh):
                    j, qh = pairs[pi]
                    ro = 64 * hh
                    av = av_t.pop((pi, hh))
                    rep = nrm.tile([DH, 2, 512], F32, tag="rep")
                    # sums row (psum partition 64) -> partition 0 SBUF on
                    # DVE (ScalarE is saturated with exps), then gpsimd-
                    # broadcast to 64 partitions, then approx-recip there
                    # (the custom DVE op misbehaves at base >= 64).
                    sums = nrm.tile([1, 512], F32, tag="sums")
                    if pi == len(pairs) - 1:
                        # ScalarE is free once the last exps retire; taking
                        # the sums copy there shortens the serialized DVE
                        # chain that gates the final out-proj units
                        nc.scalar.activation(out=sums[:], in_=av[DH:DH + 1, :],
                                             func=AF.Copy)
                    else:
                        nc.vector.tensor_copy(sums[:], av[DH:DH + 1, :])
                    # raw-av copy frees the psum bank ~1.4us after the last
                    # av matmul (instead of after the whole 3us normalize
                    # chain) so the next pair's av allocation never stalls.
                    avr = nrm.tile([DH, 512], F32, tag="avr", bufs=3)
                    nc.vector.tensor_copy(avr[:], av[0:DH, :])
                    nc.gpsimd.partition_broadcast(
                        rep[:, 0, :], sums[:], channels=DH)
                    nc.vector.reciprocal_approx_fast(
                        out=rep[:, 1, :], in_=rep[:, 0, :])
                    nc.vector.tensor_tensor(
                        out=outT_sb[ro:ro + DH, j, qh * 512:(qh + 1) * 512],
                        in0=avr[:], in1=rep[:, 1, :], op=ALU.mult)
                    if debug and pi == 0 and hh == 0:
                        avc = nrm.tile([DH + 1, 512], F32, tag="avc")
                        nc.vector.tensor_copy(avc[:], av[:])
                        nc.sync.dma_start(out=dbg["d_av"][:, :], in_=avc)
                        nc.sync.dma_start(out=dbg["d_rcp"][:, :],
                                          in_=rep[0:1, 1, :])
                        nc.sync.dma_start(out=dbg["d_rep"][:, :],
                                          in_=rep[:, 1, :])

                # qh0 units (t 0..3) run fully as attention fillers. qh1
                # units (t 4..7) run in two stages: cc 0..2 partial-summed
                # to SBUF f32 during the last pair (j0..2 finales are in),
                # then one cc=3 matmul + DVE add after the last finale.
                proj_units = [(t, oh) for t in range(NT) for oh in range(2)]
                stg_t = {}
                prt = {}
                state = {"emitted": 0, "finales": 0, "vdef": 0, "partial": 8}

                def unit_mms(pp, t, oh, ccs, start_cc, stop_cc):
                    for cc in ccs:
                        nc.tensor.matmul(
                            pp[:],
                            outT_sb[:, cc, t * 128:(t + 1) * 128],
                            wproj_sb[:, cc, oh * 512:(oh + 1) * 512],
                            start=(cc == start_cc), stop=(cc == stop_cc))

                def emit_proj_unit():
                    t, oh = proj_units[state["emitted"]]
                    state["emitted"] += 1
                    if oh == 0:
                        stg_t[t] = outp.tile([128, C], BF16, name=f"stg{t}",
                                             tag="stg")
                    pp = psP.tile([128, 512], F32, tag="pp2")
                    unit_mms(pp, t, oh, range(4), 0, 3)
                    nc.vector.tensor_copy(
                        stg_t[t][:, oh * 512:(oh + 1) * 512], pp[:])
                    if oh == 1:
                        nc.sync.dma_start(out=out[t * 128:(t + 1) * 128, :],
                                          in_=stg_t.pop(t))

                def emit_partial_unit():
                    # t=4,5 units: cc0..2 partial -> SBUF f32 (final = DVE add)
                    t, oh = proj_units[state["partial"]]
                    state["partial"] += 1
                    pp = psP.tile([128, 512], F32, tag="pp2")
                    unit_mms(pp, t, oh, range(3), 0, 2)
                    prt[(t, oh)] = outp.tile([128, 512], F32,
                                             name=f"prt{t}_{oh}", tag="prt",
                                             bufs=4)
                    nc.vector.tensor_copy(prt[(t, oh)][:], pp[:])

                prt_ps = {}

                def emit_psum_partials():
                    # t=6,7 units: cc0..2 stay resident in psS tiles (their
                    # sc traffic is over); final = one accumulating matmul +
                    # a ScalarE copy, so the tail splits across DVE + ScalarE
                    for t in (6, 7):
                        pt = psS.tile([128, 2, 512], F32, name=f"prtps{t}",
                                      tag="sc", bufs=2)
                        for oh in range(2):
                            unit_mms(pt[:, oh, :], t, oh, range(3), 0, -1)
                        prt_ps[t] = pt

                def emit_final_unit(t, oh):
                    if oh == 0:
                        stg_t[t] = outp.tile([128, C], BF16, name=f"stg{t}",
                                             tag="stg")
                    if t in prt_ps:
                        pt = prt_ps[t]
                        unit_mms(pt[:, oh, :], t, oh, [3], -1, 3)
                        nc.scalar.activation(
                            out=stg_t[t][:, oh * 512:(oh + 1) * 512],
                            in_=pt[:, oh, :], func=AF.Copy)
                    else:
                        # the av banks are all retired by now — rotating the
                        # final-unit psums through psV avoids serializing on
                        # the single psP bank
                        pp = psV.tile([128, 512], F32, tag="av",
                                      name=f"fu{t}_{oh}")
                        unit_mms(pp, t, oh, [3], 3, 3)
                        nc.vector.tensor_tensor(
                            out=stg_t[t][:, oh * 512:(oh + 1) * 512],
                            in0=pp[:], in1=prt.pop((t, oh))[:], op=ALU.add)
                    if oh == 1:
                        nc.sync.dma_start(out=out[t * 128:(t + 1) * 128, :],
                                          in_=stg_t.pop(t))

                # deferred v tiles 4-7 spread over attention steps 0-5;
                # deferred v tiles 4-7 spread as half-tile fillers over the
                # first chunk-steps; tile T's second half lands at step
                # 2(T-4)+1, well before its first av reader (kc=T) is
                # emitted at step T+6 with the lookahead of 6.
                vplan = [(6, 0), (6, 1), (7, 0), (7, 1)]
                vps = {}

                def emit_filler():
                    if state["vdef"] < len(vplan):
                        t, half = vplan[state["vdef"]]
                        state["vdef"] += 1
                        if half == 0:
                            vps[t] = psP.tile([128, 512], F32,
                                              name=f"vt{t}", tag="pp2")
                            proj_mms(vps[t], 2, t, range(0, KC // 2))
                        else:
                            proj_mms(vps[t], 2, t, range(KC // 2, KC))
                            post_v(vps.pop(t), t, on_dve=True)
                        return
                    if state["finales"] >= 8 and state["emitted"] < 8:
                        emit_proj_unit()
                        return
                    if state["finales"] >= 14 and state["partial"] < 12:
                        emit_partial_unit()
                        if state["partial"] < 12:
                            emit_partial_unit()

                # pipeline: sc/exp run 6 chunk-steps ahead of av; finales
                # (all DVE/GpSimd) are emitted as soon as the last av lands.
                steps = [(pi, kc) for pi in range(len(pairs))
                         for kc in range(NT)]
                exq = []      # (pi, kc, ex) awaiting av emission
                fill_tick = 0
                for (pi, kc) in steps:
                    # last pair: drop the av lookahead to 1 so the drain
                    # chain (av -> finale -> out-proj tail) starts sooner
                    look = 1 if pi == len(pairs) - 1 else 7
                    while len(exq) >= look:
                        api, akc, aex = exq.pop(0)
                        emit_av_pair(api, akc, aex)
                        if akc == NT - 1:
                            emit_finale(api, 0)
                            emit_finale(api, 1)
                            state["finales"] += 2
                    exq.append((pi, kc, emit_sc_pair(pi, kc)))
                    # fillers are ~1.7-1us of PE work; one per two chunk-
                    # steps keeps the PE just above the exp pace
                    fill_tick += 1
                    if state["vdef"] < len(vplan) or fill_tick % 2 == 0:
                        emit_filler()
                for (api, akc, aex) in exq:
                    emit_av_pair(api, akc, aex)
                    if akc == NT - 1:
                        emit_finale(api, 0)
                        emit_finale(api, 1)
                        state["finales"] += 2
                if debug:
                    nc.sync.dma_start(out=dbg["d_outT"][:, :],
                                      in_=outT_sb[:, 0, :])
                    nc.sync.dma_start(
                        out=dbg["d_v"][:, :].rearrange(
                            "(t p) (h d) -> p t h d", p=128, h=HPC),
                        in_=v_sb)
                while state["emitted"] < 8:
                    emit_proj_unit()
                while state["partial"] < 12:
                    emit_partial_unit()
                emit_psum_partials()
                # interleave DVE-add units (t4,5) with ScalarE-copy units
                # (t6,7) so the tail splits across both engines
                for t in (4, 6, 5, 7):
                    emit_final_unit(t, 0)
                    emit_final_unit(t, 1)

    nc.finalize()
    return nc


_CACHE = {}
_LAST_RES = None


def _bf16(a):
    return np.ascontiguousarray(a.astype(ml_dtypes.bfloat16))


def kernel(x, rope, delta_t_emb, v_residual_v1, Wqkv, bqkv, Wdt, bdt,
           qn_g, qn_b, kn_g, kn_b, lamb1, lamb2, Wproj, bproj):
    x = np.asarray(x, np.float32)
    rope = np.ascontiguousarray(np.asarray(rope, np.float32))
    delta_t_emb = np.asarray(delta_t_emb, np.float32)
    v_residual_v1 = np.asarray(v_residual_v1, np.float32)
    Wqkv = np.asarray(Wqkv, np.float32)
    Wdt = np.asarray(Wdt, np.float32)
    Wproj = np.asarray(Wproj, np.float32)
    bias = np.asarray(bqkv, np.float32) + np.asarray(bdt, np.float32)
    l1 = float(np.asarray(lamb1)); l2 = float(np.asarray(lamb2))
    qn_g = np.asarray(qn_g, np.float32); qn_b = np.asarray(qn_b, np.float32)
    kn_g = np.asarray(kn_g, np.float32); kn_b = np.asarray(kn_b, np.float32)

    has_bias = bool(np.any(bias))
    has_ln = not (np.all(qn_g == 1.0) and np.all(qn_b == 0.0)
                  and np.all(kn_g == 1.0) and np.all(kn_b == 0.0))

    dbgf = bool(int(os.environ.get("KERNEL_DEBUG", "0")))
    key = (l1, has_bias, has_ln, dbgf)
    if key not in _CACHE:
        _CACHE[key] = build(l1, has_bias, has_ln, debug=dbgf)
    nc = _CACHE[key]

    # host-prepared rope tables in SBUF layout [p, t*DH]:
    # cos table and sign-folded sin table (rotate_half absorbed:
    # out = x*cos + rot(x)*sin' with sin' = [-sin_lo || sin_hi]).
    sin = rope[:, 0:DH]; cos = rope[:, DH:2 * DH]
    sinp = np.concatenate([-sin[:, 0:HD], sin[:, HD:DH]], axis=1)

    def _ptile(a):  # [N, DH] -> [128, NT*DH] with n = t*128 + p
        return _bf16(a.reshape(NT, 128, DH).transpose(1, 0, 2).reshape(128, -1))

    cos_p = _ptile(cos)
    sin_p = _ptile(sinp)

    in_maps = []
    for c in range(8):
        b = c // 2
        g = c % 2
        rsl = slice(g * 512, (g + 1) * 512)
        w_core = np.concatenate([
            np.concatenate([Wqkv[rsl], Wqkv[C:][rsl], Wqkv[2 * C:][rsl]], 0).T,
            np.concatenate([Wdt[rsl], Wdt[C:][rsl], Wdt[2 * C:][rsl]], 0).T,
        ], axis=0)
        w_core = np.ascontiguousarray(w_core)
        bc = np.concatenate([bias[rsl], bias[C:][rsl], bias[2 * C:][rsl]])
        bc = bc.astype(np.float32).copy()
        # fold LN mean-centering into the q/k weight+bias head blocks
        # (exact: (x@W + b)@C = x@(W@C) + b@C with C = I - J/64)
        for ob in range(2):
            for h in range(HPC):
                sl = slice(ob * 512 + h * DH, ob * 512 + (h + 1) * DH)
                w_core[:, sl] -= w_core[:, sl].mean(axis=1, keepdims=True)
                bc[sl] -= bc[sl].mean()
        # vres in SBUF layout [p, t, h, d] flattened
        vr = (l2 * v_residual_v1[b, g * 8:(g + 1) * 8]).transpose(1, 0, 2)
        vr = vr.reshape(NT, 128, HPC, DH).transpose(1, 0, 2, 3).reshape(128, -1)
        # rotate each core's contraction-chunk order (sum order is free):
        # the 8 SPMD cores otherwise request the SAME chunk simultaneously
        # and serialize on HBM during the critical early stream
        rot = (2 * c) % KC
        perm = [(k + rot) % KC for k in range(KC)]
        xdT_full = np.concatenate([x[b].T, delta_t_emb[b].T], 0)
        xdT_rot = np.concatenate([xdT_full[128 * k:128 * (k + 1)]
                                  for k in perm], 0)
        w_rot = np.concatenate([w_core[128 * k:128 * (k + 1)]
                                for k in perm], 0)
        m = {
            "xdT": _bf16(xdT_rot),
            "w": _bf16(w_rot),
            "vres": _bf16(vr),
            "ropec": cos_p,
            "ropes": sin_p,
            "wproj": _bf16(Wproj[:, rsl].T),
        }
        if has_bias:
            m["biasd"] = np.ascontiguousarray(bc[None, :].astype(np.float32))
        if has_ln:
            m["lnp"] = _bf16(np.stack([qn_g, qn_b, kn_g, kn_b], 0))
        in_maps.append(m)

    trace = bool(int(os.environ.get("KERNEL_TRACE", "0")))
    res = run_bass_kernel_spmd(nc, in_maps, core_ids=list(range(8)), trace=trace)
    global _LAST_RES
    _LAST_RES = res
    if trace and res.exec_time_ns is not None:
        print(f"HW exec time: {res.exec_time_ns} ns")
        kernel.last_exec_time_ns = res.exec_time_ns
        kernel.last_results = res

    out = np.empty((B, N, C), np.float32)
    for b in range(B):
        out[b] = (res.results[2 * b]["out"].astype(np.float32)
                  + res.results[2 * b + 1]["out"].astype(np.float32))
    bproj = np.asarray(bproj, np.float32)
    if np.any(bproj):
        out += bproj[None, None, :]
    return out
